# revision 7
# baseline (speedup 1.0000x reference)
"""Bidirectional LSTM on trn2 NeuronCores.

Sharding: 2 cores, one per direction, full batch B=32 per core. The
backward core receives time-reversed x and its output is re-reversed on
the host. The scan is fully core-local (the recurrence never crosses the
wire), and using 2 cores instead of 8 minimizes total device-seconds:
the scan cost is dominated by W_hh stationary-weight ingestion into the
PE array (64 LDWEIGHTS x 128x128 fp16 per step), which is independent of
the per-core batch size, so batch-splitting across more cores multiplies
device time without reducing latency.

Per-core plan (B=32, T=512, I=256, H=512, G=4H=2048):
  1. Host pre-transposes/casts weights and x to fp16 (lhsT / moving
     layouts, t-major x). Gate blocks permuted to i,f,o,g so sigmoid
     covers one contiguous 96-col span per chunk.
  2. Phase C: xp = x @ W_ih.T + b for all T, written to a DRAM buffer in
     16-step window layout (doesn't fit SBUF at B=32), N=512 moving
     columns per matmul so LDWEIGHTS is fully amortized.
  3. 512-step scan, chunk-pipelined: gates are computed per H-chunk k
     (16 matmuls -> PSUM tile [128, 4x32]), and each chunk's
     DVE/ACT tail (add xp, sigmoid/tanh, cell update) runs while the PE
     streams the next chunk's weights. h chunks are written straight
     into the fp16 output window tile, which doubles as next step's
     moving operand, so the serial tail at a step boundary is one chunk
     deep instead of a full step.
  4. xp windows stream DRAM->SBUF double-buffered; output windows
     (16 steps) DMA out as they complete; host unscrambles + upcasts.

The compiled PJRT executable is cached at module level: repeat kernel()
calls only transfer fresh inputs and execute.
"""

import numpy as np

B_FULL, T, I, H = 32, 512, 256, 512
G = 4 * H
N_CORES = 2
B = B_FULL                # per-core batch (one direction per core)
KH = H // 128             # 4 contraction chunks for W_hh
KI = I // 128             # 2 contraction chunks for W_ih
M = G // 128              # 16 gate-row chunks (4 per gate)
WIN = 16                  # scan steps per xp/output window
TB = T * B                # 16384 moving columns, t-major
WCOL = M * WIN * B        # 8192 xp columns per window
T_SCAN = T

_BUILT = {}


def _install_tile_patch():
    """This container's walrus accepts only ONE sync-wait per CTRL-class
    instruction (Drain/NoOp). Tile's kernel-tail drain aggregates one wait
    per semaphore lane onto a single Drain -> split them one per drain."""
    import bass_rust
    import concourse.tile as tile

    if getattr(tile.TileContext, "_drain_split_patched", False):
        return

    def _patched_dab(self, tick_clock, wait_clock):
        from concourse.tile import ScopedClock

        nc = self.nc
        drain_inst = nc.sync.drain()
        wait_clock.add_sem_waits(
            drain_inst.ins, ScopedClock({None: tick_clock.global_clock})
        )
        si = drain_inst.ins.sync_info
        waits = list(si.on_wait) if si is not None else []
        if len(waits) > 1:
            si.on_wait = waits[:1]
            for w in waits[1:]:
                d2 = nc.sync.drain()
                si2 = d2.ins.sync_info
                if si2 is None:
                    d2.ins.sync_info = bass_rust.SyncInfo(on_wait=[w], on_update=[])
                else:
                    si2.on_wait = list(si2.on_wait) + [w]
        nc.all_engine_barrier()
        assert self.sems is not None
        popped = nc._tile_sem_poison_stack.pop()
        assert popped is self._sem_poison
        nc.clear_and_free_semaphores(list(self.sems.allocated().values()))
        nc.all_engine_barrier()

    tile.TileContext._drain_and_barrier = _patched_dab
    tile.TileContext._drain_split_patched = True

    # This walrus build accepts at most ONE sync-wait per instruction (any
    # opcode). Split every multi-wait instruction at BIR-JSON level into
    # single-wait NoOps followed by the real instruction with one wait.
    import json
    import concourse.bass as bass

    if getattr(bass.Bass, "_json_wait_split_patched", False):
        return
    _orig_tjb = bass.Bass.to_json_bytes

    def _split_json(self):
        raw = _orig_tjb(self)
        m = json.loads(raw)
        ctr = 0
        changed = False
        for fn in m.get("functions", []):
            for bb in fn.get("blocks", []):
                out = []
                for inst in bb.get("instructions", []):
                    si = inst.get("sync_info")
                    waits = (si or {}).get("on_wait") or []
                    if len(waits) > 1:
                        changed = True
                        for w in waits[:-1]:
                            ctr += 1
                            nop = {
                                "engine": inst["engine"],
                                "ins": [],
                                "outs": [],
                                "name": f"WSPLIT-{ctr}",
                                "opcode": "NoOp",
                                "sync_info": {"on_update": [], "on_wait": [w]},
                            }
                            if "debug" in inst:
                                nop["debug"] = inst["debug"]
                            out.append(nop)
                        si["on_wait"] = [waits[-1]]
                    out.append(inst)
                bb["instructions"] = out
        if not changed:
            return raw
        return json.dumps(m).encode()

    bass.Bass.to_json_bytes = _split_json
    bass.Bass._json_wait_split_patched = True


def _build(t_scan):
    import concourse.bass as bass
    import concourse.tile as tile
    from concourse import mybir
    from contextlib import ExitStack

    _install_tile_patch()
    f32 = mybir.dt.float32
    f16 = mybir.dt.float16

    assert t_scan % WIN == 0
    n_win = t_scan // WIN

    nc = bass.Bass()
    # Host layouts: xT [I, T*B] f16 t-major (col t*B + b), wihT [I, G] f16,
    # whhT [H, G] f16 (G rows permuted to gate order i,f,o,g; g-gate rows
    # pre-scaled by 2 so tanh(x) = 2*sigmoid(2x)-1 folds into the single
    # sigmoid pass), bsb [128, M], ident = eye(128) f16.
    xt_d = nc.dram_tensor("xT", [I, TB], f16, kind="ExternalInput")
    wiht_d = nc.dram_tensor("wihT", [I, G], f16, kind="ExternalInput")
    whht_d = nc.dram_tensor("whhT", [H, G], f16, kind="ExternalInput")
    bsb_d = nc.dram_tensor("bsb", [128, M], f32, kind="ExternalInput")
    id_d = nc.dram_tensor("ident", [128, 128], f16, kind="ExternalInput")
    # out[w, p, s*128 + k*32 + b] = h[b, 16w+s, 128k+p]
    out_d = nc.dram_tensor("out_raw", [n_win, 128, WIN * KH * B], f16,
                           kind="ExternalOutput")

    with tile.TileContext(nc) as tc, ExitStack() as ctx:
        sig = mybir.ActivationFunctionType.Sigmoid
        tanh = mybir.ActivationFunctionType.Tanh

        wpool = ctx.enter_context(tc.tile_pool(name="w", bufs=1))
        dpool = ctx.enter_context(tc.tile_pool(name="d", bufs=1, space="DRAM"))
        whhT = wpool.tile([128, KH * G], f16)    # tile (kk,m) at (kk*M+m)*128
        wihT = wpool.tile([128, KI * G], f16)
        xT = wpool.tile([128, KI * TB], f16)     # chunk ki at ki*TB
        b_sb = wpool.tile([128, M], f32)
        ident = wpool.tile([128, 128], f16)
        # xp DRAM buffer, window layout: col w*WCOL + m*(WIN*B) + s*B + b
        xp_dram = dpool.tile([128, n_win * WCOL], f16)
        nc.gpsimd.dma_start(b_sb[:], bsb_d[:])
        nc.gpsimd.dma_start(ident[:], id_d[:])
        for k in range(KH):
            nc.gpsimd.dma_start(whhT[:, k * G:(k + 1) * G],
                                whht_d[k * 128:(k + 1) * 128, :])
        for k in range(KI):
            nc.gpsimd.dma_start(wihT[:, k * G:(k + 1) * G],
                                wiht_d[k * 128:(k + 1) * 128, :])
            nc.gpsimd.dma_start(xT[:, k * TB:(k + 1) * TB],
                                xt_d[k * 128:(k + 1) * 128, :])

        # ---- phase C: xp = x @ W_ih.T + b -> DRAM, fp16 ----
        NXP = WIN * B  # 512 moving columns = one window of one m-chunk
        with tc.tile_pool(name="xpps", bufs=4, space="PSUM") as xpp, \
             tc.tile_pool(name="xpsb", bufs=3) as xsb:
            for w in range(n_win):
                for mq in range(M // 4):
                    sb = xsb.tile([128, 4 * NXP], f16, tag="xsb")
                    for mi in range(4):
                        m = mq * 4 + mi
                        ps = xpp.tile([128, NXP], f32, tag="xps")
                        for k in range(KI):
                            nc.tensor.matmul(
                                ps[:],
                                wihT[:, (k * M + m) * 128:(k * M + m + 1) * 128],
                                xT[:, k * TB + w * NXP:k * TB + (w + 1) * NXP],
                                start=(k == 0), stop=(k == KI - 1),
                            )
                        dst = sb[:, mi * NXP:(mi + 1) * NXP]
                        if m % 2 == 0:
                            nc.vector.tensor_scalar_add(dst, ps[:],
                                                        b_sb[:, m:m + 1])
                        else:
                            nc.scalar.add(dst, ps[:], b_sb[:, m:m + 1])
                    nc.gpsimd.dma_start(
                        xp_dram[:, w * WCOL + mq * 4 * NXP:
                                w * WCOL + (mq + 1) * 4 * NXP],
                        sb[:])

        # ---- phase D: the scan ----
        # gate m-chunk = g*4 + k (g in i,f,o,g order; k = H 128-chunk)
        # h/c col layout: k*32 + b
        with tc.tile_pool(name="gp", bufs=8, space="PSUM") as gp, \
             tc.tile_pool(name="xpw", bufs=2) as xpool, \
             tc.tile_pool(name="acts", bufs=4) as ap, \
             tc.tile_pool(name="state", bufs=2) as stp, \
             tc.tile_pool(name="outb", bufs=2) as obp, \
             tc.tile_pool(name="init", bufs=1) as ip:
            h0 = ip.tile([128, KH * B], f16)
            c0 = ip.tile([128, KH * B], f32)
            nc.vector.memset(h0[:], 0.0)
            nc.vector.memset(c0[:], 0.0)

            def load_window(w):
                tl = xpool.tile([128, WCOL], f16, tag="xp")
                nc.gpsimd.dma_start(tl[:], xp_dram[:, w * WCOL:(w + 1) * WCOL])
                return tl

            xpw_cur = load_window(0)
            xpw_next = None
            h_src = h0
            c_prev = c0
            ob = None
            for t in range(t_scan):
                w, s = divmod(t, WIN)
                if s == 0:
                    if w > 0:
                        xpw_cur = xpw_next
                    if w + 1 < n_win:
                        xpw_next = load_window(w + 1)
                    ob = obp.tile([128, WIN * KH * B], f16, tag="ob")
                # xp view: [p, g, k, s, b]
                xp5 = xpw_cur.rearrange("p (g k s b) -> p g k s b",
                                        g=4, k=KH, s=WIN)
                c_t = stp.tile([128, KH * B], f32, tag="c")
                for k in range(KH):
                    ps = gp.tile([128, 4 * B], f32, tag="ps")  # i|f|o|g~
                    # preload xp into PSUM (identity matmul, 3D moving AP)
                    nc.tensor.matmul(ps[:], ident[:], xp5[:, :, k, s, :],
                                     start=True, stop=False)
                    for g in range(4):
                        m = g * KH + k
                        for kk in range(KH):
                            nc.tensor.matmul(
                                ps[:, g * B:(g + 1) * B],
                                whhT[:, (kk * M + m) * 128:(kk * M + m + 1) * 128],
                                h_src[:, kk * B:(kk + 1) * B],
                                start=False, stop=(kk == KH - 1),
                            )
                    # tail for chunk k: af = sigmoid over all 4 blocks;
                    # g~ = sigmoid(2*g_pre) (host pre-scaled), so
                    # i*g = 2*(g~ - 0.5)*i and c = 2*q + f*c_prev.
                    af = ap.tile([128, 4 * B], f32, tag="af")
                    nc.scalar.activation(af[:], ps[:], sig)
                    q = ap.tile([128, B], f32, tag="q")
                    nc.vector.scalar_tensor_tensor(
                        q[:], af[:, 3 * B:4 * B], 0.5, af[:, 0:B],
                        op0=mybir.AluOpType.subtract, op1=mybir.AluOpType.mult)
                    fc = ap.tile([128, B], f32, tag="fc")
                    nc.gpsimd.tensor_mul(fc[:], af[:, B:2 * B],
                                         c_prev[:, k * B:(k + 1) * B])
                    nc.vector.scalar_tensor_tensor(
                        c_t[:, k * B:(k + 1) * B], q[:], 2.0, fc[:],
                        op0=mybir.AluOpType.mult, op1=mybir.AluOpType.add)
                    th = ap.tile([128, B], f32, tag="th")
                    nc.scalar.activation(th[:], c_t[:, k * B:(k + 1) * B], tanh)
                    nc.vector.tensor_mul(
                        ob[:, s * KH * B + k * B:s * KH * B + (k + 1) * B],
                        af[:, 2 * B:3 * B], th[:])
                h_src = ob[:, s * KH * B:(s + 1) * KH * B]
                c_prev = c_t
                if s == WIN - 1:
                    nc.gpsimd.dma_start(out_d[w], ob[:])

    return nc


def _get_nc(t_scan):
    key = t_scan
    if key not in _BUILT:
        _BUILT[key] = _build(t_scan)
    return _BUILT[key]


_RUNNERS = {}


def _make_runner(t_scan):
    """Compile once, return a callable in_maps -> list[dict] that only
    executes (PJRT executable cached across kernel() calls)."""
    import jax
    import jax.numpy as jnp
    import numpy as np
    from jax.sharding import Mesh, PartitionSpec
    from jax.experimental.shard_map import shard_map
    from concourse import bass2jax, mybir
    from concourse.bass2jax import _bass_exec_p, install_neuronx_cc_hook

    install_neuronx_cc_hook()
    nc = _get_nc(t_scan)
    assert nc.dbg_addr is None
    n_cores = N_CORES
    partition_name = (nc.partition_id_tensor.name
                      if nc.partition_id_tensor else None)
    in_names, out_names, out_avals, zero_shapes = [], [], [], []
    for alloc in nc.m.functions[0].allocations:
        if not isinstance(alloc, mybir.MemoryLocationSet):
            continue
        name = alloc.memorylocations[0].name
        if alloc.kind == "ExternalInput":
            if name != partition_name:
                in_names.append(name)
        elif alloc.kind == "ExternalOutput":
            shape = tuple(alloc.tensor_shape)
            npdt = mybir.dt.np(alloc.dtype)
            out_avals.append(jax.core.ShapedArray(shape, npdt))
            out_names.append(name)
            zero_shapes.append((shape, npdt))
    n_params = len(in_names)
    n_outs = len(out_names)
    all_in = in_names + out_names
    if partition_name is not None:
        all_in = all_in + [partition_name]

    def _body(*args):
        operands = list(args)
        if partition_name is not None:
            operands.append(bass2jax.partition_id_tensor())
        outs = _bass_exec_p.bind(
            *operands,
            out_avals=tuple(out_avals),
            in_names=tuple(all_in),
            out_names=tuple(out_names),
            lowering_input_output_aliases=(),
            sim_require_finite=True,
            sim_require_nnan=True,
            nc=nc,
        )
        return tuple(outs)

    devices = jax.devices()[:n_cores]
    mesh = Mesh(np.asarray(devices), ("core",))
    donate = tuple(range(n_params, n_params + n_outs))
    sharded = jax.jit(
        shard_map(_body, mesh=mesh,
                  in_specs=(PartitionSpec("core"),) * (n_params + n_outs),
                  out_specs=(PartitionSpec("core"),) * n_outs,
                  check_rep=False),
        donate_argnums=donate, keep_unused=True,
    )

    def run(in_maps):
        concat_in = [
            np.concatenate([np.asarray(m[name]) for m in in_maps], axis=0)
            for name in in_names
        ]
        concat_zeros = [
            jnp.zeros((n_cores * s[0], *s[1:]), dt) for s, dt in zero_shapes
        ]
        out_arrs = sharded(*concat_in, *concat_zeros)
        return [
            {name: np.asarray(out_arrs[i]).reshape(
                n_cores, *out_avals[i].shape)[c]
             for i, name in enumerate(out_names)}
            for c in range(n_cores)
        ]

    run.in_names = in_names
    run.out_names = out_names
    run.zero_shapes = zero_shapes
    run.sharded = sharded
    run.n_cores = n_cores
    return run


def _get_runner(t_scan):
    if t_scan not in _RUNNERS:
        _RUNNERS[t_scan] = _make_runner(t_scan)
    return _RUNNERS[t_scan]


_GATE_PERM = None


def _gate_perm():
    global _GATE_PERM
    if _GATE_PERM is None:
        # reference gate row order i,f,g,o -> kernel order i,f,o,g
        _GATE_PERM = np.concatenate([
            np.arange(0, H), np.arange(H, 2 * H),
            np.arange(3 * H, 4 * H), np.arange(2 * H, 3 * H)])
    return _GATE_PERM


def make_in_maps(x, W_ih_f, W_hh_f, b_f, W_ih_b, W_hh_b, b_b):
    """Host-side input prep: one map per core (0=forward, 1=backward)."""
    x = np.asarray(x, dtype=np.float32)
    perm = _gate_perm()
    in_maps = []
    ident = np.eye(128, dtype=np.float16)
    for d, (wih, whh, bb) in enumerate(
            [(W_ih_f, W_hh_f, b_f), (W_ih_b, W_hh_b, b_b)]):
        wihp = np.asarray(wih, np.float32)[perm].copy()
        whhp = np.asarray(whh, np.float32)[perm].copy()
        bp = np.asarray(bb, np.float32)[perm].copy()
        # pre-scale g-gate rows by 2: tanh(x) = 2*sigmoid(2x) - 1
        wihp[3 * H:] *= 2.0
        whhp[3 * H:] *= 2.0
        bp[3 * H:] *= 2.0
        xd = x if d == 0 else x[:, ::-1]
        xt = np.ascontiguousarray(
            xd.transpose(2, 1, 0).reshape(I, TB)).astype(np.float16)
        in_maps.append({
            "xT": xt,
            "wihT": np.ascontiguousarray(wihp.T).astype(np.float16),
            "whhT": np.ascontiguousarray(whhp.T).astype(np.float16),
            "bsb": np.ascontiguousarray(bp.reshape(M, 128).T),
            "ident": ident,
        })
    return in_maps


def unscramble(results, t_scan):
    """results: list of per-core out dicts -> full [32, t_scan, 1024]."""
    n_win = t_scan // WIN
    halves = []
    for d in range(2):
        raw = np.asarray(results[d]["out_raw"])  # [n_win, 128, WIN*KH*B]
        h = raw.reshape(n_win, 128, WIN, KH, B)
        h = np.ascontiguousarray(h.transpose(4, 0, 2, 3, 1))
        h = h.reshape(B, n_win * WIN, H)[:, :t_scan]
        if d == 1:
            h = h[:, ::-1]
        halves.append(h)
    return np.concatenate(halves, axis=2).astype(np.float32)


def kernel(x, W_ih_f, W_hh_f, b_f, W_ih_b, W_hh_b, b_b, _t_scan=T_SCAN):
    in_maps = make_in_maps(x, W_ih_f, W_hh_f, b_f, W_ih_b, W_hh_b, b_b)
    try:
        runner = _get_runner(_t_scan)
        results = runner(in_maps)
    except Exception:
        from concourse.bass_utils import run_bass_kernel_spmd
        res = run_bass_kernel_spmd(_get_nc(_t_scan), in_maps,
                                   list(range(N_CORES)))
        results = res.results
    return unscramble(results, _t_scan)


# revision 9
# speedup vs baseline: 1.1402x; 1.1402x over previous
"""Bidirectional LSTM on trn2 NeuronCores.

Sharding: 2 cores, one per direction, full batch B=32 per core. The
backward core receives time-reversed x and its output is re-reversed on
the host. The scan is fully core-local (the recurrence never crosses the
wire), and using 2 cores instead of 8 minimizes total device-seconds:
the scan cost is dominated by W_hh stationary-weight ingestion into the
PE array (64 LDWEIGHTS x 128x128 fp16 per step), which is independent of
the per-core batch size, so batch-splitting across more cores multiplies
device time without reducing latency.

Per-core plan (B=32, T=512, I=256, H=512, G=4H=2048):
  1. Host pre-transposes/casts weights and x to fp16 (lhsT / moving
     layouts, t-major x). Gate blocks permuted to i,f,o,g so sigmoid
     covers one contiguous 96-col span per chunk.
  2. Phase C: xp = x @ W_ih.T + b for all T, written to a DRAM buffer in
     16-step window layout (doesn't fit SBUF at B=32), N=512 moving
     columns per matmul so LDWEIGHTS is fully amortized.
  3. 512-step scan, chunk-pipelined: gates are computed per H-chunk k
     (16 matmuls -> PSUM tile [128, 4x32]), and each chunk's
     DVE/ACT tail (add xp, sigmoid/tanh, cell update) runs while the PE
     streams the next chunk's weights. h chunks are written straight
     into the fp16 output window tile, which doubles as next step's
     moving operand, so the serial tail at a step boundary is one chunk
     deep instead of a full step.
  4. xp windows stream DRAM->SBUF double-buffered; output windows
     (16 steps) DMA out as they complete; host unscrambles + upcasts.

The compiled PJRT executable is cached at module level: repeat kernel()
calls only transfer fresh inputs and execute.
"""

import numpy as np

B_FULL, T, I, H = 32, 512, 256, 512
G = 4 * H
N_CORES = 2
B = B_FULL                # per-core batch (one direction per core)
KH = H // 128             # 4 contraction chunks for W_hh
KI = I // 128             # 2 contraction chunks for W_ih
M = G // 128              # 16 gate-row chunks (4 per gate)
WIN = 16                  # scan steps per xp/output window
TB = T * B                # 16384 moving columns, t-major
WCOL = M * WIN * B        # 8192 xp columns per window
T_SCAN = T

_BUILT = {}


def _install_tile_patch():
    """This container's walrus accepts only ONE sync-wait per CTRL-class
    instruction (Drain/NoOp). Tile's kernel-tail drain aggregates one wait
    per semaphore lane onto a single Drain -> split them one per drain."""
    import bass_rust
    import concourse.tile as tile

    if getattr(tile.TileContext, "_drain_split_patched", False):
        return

    def _patched_dab(self, tick_clock, wait_clock):
        from concourse.tile import ScopedClock

        nc = self.nc
        drain_inst = nc.sync.drain()
        wait_clock.add_sem_waits(
            drain_inst.ins, ScopedClock({None: tick_clock.global_clock})
        )
        si = drain_inst.ins.sync_info
        waits = list(si.on_wait) if si is not None else []
        if len(waits) > 1:
            si.on_wait = waits[:1]
            for w in waits[1:]:
                d2 = nc.sync.drain()
                si2 = d2.ins.sync_info
                if si2 is None:
                    d2.ins.sync_info = bass_rust.SyncInfo(on_wait=[w], on_update=[])
                else:
                    si2.on_wait = list(si2.on_wait) + [w]
        nc.all_engine_barrier()
        assert self.sems is not None
        popped = nc._tile_sem_poison_stack.pop()
        assert popped is self._sem_poison
        nc.clear_and_free_semaphores(list(self.sems.allocated().values()))
        nc.all_engine_barrier()

    tile.TileContext._drain_and_barrier = _patched_dab
    tile.TileContext._drain_split_patched = True

    # This walrus build accepts at most ONE sync-wait per instruction (any
    # opcode). Split every multi-wait instruction at BIR-JSON level into
    # single-wait NoOps followed by the real instruction with one wait.
    import json
    import concourse.bass as bass

    if getattr(bass.Bass, "_json_wait_split_patched", False):
        return
    _orig_tjb = bass.Bass.to_json_bytes

    def _split_json(self):
        raw = _orig_tjb(self)
        m = json.loads(raw)
        ctr = 0
        changed = False
        for fn in m.get("functions", []):
            for bb in fn.get("blocks", []):
                out = []
                for inst in bb.get("instructions", []):
                    si = inst.get("sync_info")
                    waits = (si or {}).get("on_wait") or []
                    if len(waits) > 1:
                        changed = True
                        for w in waits[:-1]:
                            ctr += 1
                            nop = {
                                "engine": inst["engine"],
                                "ins": [],
                                "outs": [],
                                "name": f"WSPLIT-{ctr}",
                                "opcode": "NoOp",
                                "sync_info": {"on_update": [], "on_wait": [w]},
                            }
                            if "debug" in inst:
                                nop["debug"] = inst["debug"]
                            out.append(nop)
                        si["on_wait"] = [waits[-1]]
                    out.append(inst)
                bb["instructions"] = out
        if not changed:
            return raw
        return json.dumps(m).encode()

    bass.Bass.to_json_bytes = _split_json
    bass.Bass._json_wait_split_patched = True


def _build(t_scan):
    import concourse.bass as bass
    import concourse.tile as tile
    from concourse import mybir
    from contextlib import ExitStack

    _install_tile_patch()
    f32 = mybir.dt.float32
    f16 = mybir.dt.float16

    assert t_scan % WIN == 0
    n_win = t_scan // WIN

    nc = bass.Bass()
    # Host layouts: xT [I, T*B] f16 t-major (col t*B + b), wihT [I, G] f16,
    # whhT [H, G] f16 (G rows permuted to gate order i,f,o,g; g-gate rows
    # pre-scaled by 2 so tanh(x) = 2*sigmoid(2x)-1 folds into the single
    # sigmoid pass), bsb [128, M], ident = eye(128) f16.
    xt_d = nc.dram_tensor("xT", [I, TB], f16, kind="ExternalInput")
    wiht_d = nc.dram_tensor("wihT", [I, G], f16, kind="ExternalInput")
    whht_d = nc.dram_tensor("whhT", [H, G], f16, kind="ExternalInput")
    bsb_d = nc.dram_tensor("bsb", [128, M], f32, kind="ExternalInput")
    id_d = nc.dram_tensor("ident", [128, 128], f16, kind="ExternalInput")
    # out[w, p, s*128 + k*32 + b] = h[b, 16w+s, 128k+p]
    out_d = nc.dram_tensor("out_raw", [n_win, 128, WIN * KH * B], f16,
                           kind="ExternalOutput")

    with tile.TileContext(nc) as tc, ExitStack() as ctx:
        sig = mybir.ActivationFunctionType.Sigmoid
        tanh = mybir.ActivationFunctionType.Tanh

        wpool = ctx.enter_context(tc.tile_pool(name="w", bufs=1))
        dpool = ctx.enter_context(tc.tile_pool(name="d", bufs=1, space="DRAM"))
        whhT = wpool.tile([128, KH * G], f16)    # tile (kk,m) at (kk*M+m)*128
        wihT = wpool.tile([128, KI * G], f16)
        xT = wpool.tile([128, KI * TB], f16)     # chunk ki at ki*TB
        b_sb = wpool.tile([128, M], f32)
        ident = wpool.tile([128, 128], f16)
        # xp DRAM buffer, window layout: col w*WCOL + m*(WIN*B) + s*B + b
        xp_dram = dpool.tile([128, n_win * WCOL], f16)
        nc.gpsimd.dma_start(b_sb[:], bsb_d[:])
        nc.gpsimd.dma_start(ident[:], id_d[:])
        for k in range(KH):
            nc.gpsimd.dma_start(whhT[:, k * G:(k + 1) * G],
                                whht_d[k * 128:(k + 1) * 128, :])
        for k in range(KI):
            nc.gpsimd.dma_start(wihT[:, k * G:(k + 1) * G],
                                wiht_d[k * 128:(k + 1) * 128, :])
            nc.gpsimd.dma_start(xT[:, k * TB:(k + 1) * TB],
                                xt_d[k * 128:(k + 1) * 128, :])

        # ---- phase C: xp = x @ W_ih.T + b -> DRAM, fp16 ----
        NXP = WIN * B  # 512 moving columns = one window of one m-chunk
        with tc.tile_pool(name="xpps", bufs=4, space="PSUM") as xpp, \
             tc.tile_pool(name="xpsb", bufs=3) as xsb:
            for w in range(n_win):
                for mq in range(M // 4):
                    sb = xsb.tile([128, 4 * NXP], f16, tag="xsb")
                    for mi in range(4):
                        m = mq * 4 + mi
                        ps = xpp.tile([128, NXP], f32, tag="xps")
                        for k in range(KI):
                            nc.tensor.matmul(
                                ps[:],
                                wihT[:, (k * M + m) * 128:(k * M + m + 1) * 128],
                                xT[:, k * TB + w * NXP:k * TB + (w + 1) * NXP],
                                start=(k == 0), stop=(k == KI - 1),
                            )
                        dst = sb[:, mi * NXP:(mi + 1) * NXP]
                        if m % 2 == 0:
                            nc.vector.tensor_scalar_add(dst, ps[:],
                                                        b_sb[:, m:m + 1])
                        else:
                            nc.scalar.add(dst, ps[:], b_sb[:, m:m + 1])
                    nc.gpsimd.dma_start(
                        xp_dram[:, w * WCOL + mq * 4 * NXP:
                                w * WCOL + (mq + 1) * 4 * NXP],
                        sb[:])

        # ---- phase D: the scan ----
        # gate m-chunk = g*4 + k (g in i,f,o,g order; k = H 128-chunk)
        # h/c col layout: k*32 + b
        with tc.tile_pool(name="gp", bufs=8, space="PSUM") as gp, \
             tc.tile_pool(name="xpw", bufs=2) as xpool, \
             tc.tile_pool(name="acts", bufs=4) as ap, \
             tc.tile_pool(name="state", bufs=2) as stp, \
             tc.tile_pool(name="outb", bufs=2) as obp, \
             tc.tile_pool(name="init", bufs=1) as ip:
            h0 = ip.tile([128, KH * B], f16)
            c0 = ip.tile([128, KH * B], f32)
            nc.vector.memset(h0[:], 0.0)
            nc.vector.memset(c0[:], 0.0)

            def load_window(w):
                tl = xpool.tile([128, WCOL], f16, tag="xp")
                nc.gpsimd.dma_start(tl[:], xp_dram[:, w * WCOL:(w + 1) * WCOL])
                return tl

            xpw_cur = load_window(0)
            xpw_next = None
            h_src = h0
            c_prev = c0
            ob = None
            for t in range(t_scan):
                w, s = divmod(t, WIN)
                if s == 0:
                    if w > 0:
                        xpw_cur = xpw_next
                    if w + 1 < n_win:
                        xpw_next = load_window(w + 1)
                    ob = obp.tile([128, WIN * KH * B], f16, tag="ob")
                # xp view: [p, g, k, s, b]
                xp5 = xpw_cur.rearrange("p (g k s b) -> p g k s b",
                                        g=4, k=KH, s=WIN)
                c_t = stp.tile([128, KH * B], f32, tag="c")
                # all 4 xp preloads first: ready PE work at the step
                # boundary while h chunks of the previous step finish
                pss = []
                for k in range(KH):
                    ps = gp.tile([128, 4 * B], f32, tag="ps")  # i|f|o|g~
                    nc.tensor.matmul(ps[:], ident[:], xp5[:, :, k, s, :],
                                     start=True, stop=False)
                    pss.append(ps)
                # af_all[:, k*128 : (k+1)*128] = sigmoid(gates chunk k)
                af_all = ap.tile([128, KH * 4 * B], f32, tag="af")
                af4 = af_all.rearrange("p (k2 g b) -> p k2 g b", k2=KH, g=4)
                for k in range(KH):
                    ps = pss[k]
                    for g in range(4):
                        m = g * KH + k
                        for kk in range(KH):
                            nc.tensor.matmul(
                                ps[:, g * B:(g + 1) * B],
                                whhT[:, (kk * M + m) * 128:(kk * M + m + 1) * 128],
                                h_src[:, kk * B:(kk + 1) * B],
                                start=False, stop=(kk == KH - 1),
                            )
                    # tail for chunk k: af = sigmoid over all 4 blocks;
                    # g~ = sigmoid(2*g_pre) (host pre-scaled), so
                    # i*g = 2*(g~ - 0.5)*i and c = 2*q + f*c_prev.
                    af = af_all[:, k * 4 * B:(k + 1) * 4 * B]
                    nc.scalar.activation(af, ps[:], sig)
                    q = ap.tile([128, B], f32, tag="q")
                    nc.vector.scalar_tensor_tensor(
                        q[:], af[:, 3 * B:4 * B], 0.5, af[:, 0:B],
                        op0=mybir.AluOpType.subtract, op1=mybir.AluOpType.mult)
                    fc = ap.tile([128, B], f32, tag="fc")
                    nc.gpsimd.tensor_mul(fc[:], af[:, B:2 * B],
                                         c_prev[:, k * B:(k + 1) * B])
                    nc.vector.scalar_tensor_tensor(
                        c_t[:, k * B:(k + 1) * B], q[:], 2.0, fc[:],
                        op0=mybir.AluOpType.mult, op1=mybir.AluOpType.add)
                    if k % 2 == 1:
                        # paired tanh(c) + h = o * th over chunks k-1, k
                        th = ap.tile([128, 2 * B], f32, tag="th")
                        nc.scalar.activation(
                            th[:], c_t[:, (k - 1) * B:(k + 1) * B], tanh)
                        ho = ob.rearrange("p (s2 k2 b) -> p s2 k2 b",
                                          s2=WIN, k2=KH)
                        nc.vector.tensor_mul(
                            ho[:, s, k - 1:k + 1, :],
                            th.rearrange("p (k2 b) -> p k2 b", k2=2),
                            af4[:, k - 1:k + 1, 2, :],
                        )
                h_src = ob[:, s * KH * B:(s + 1) * KH * B]
                c_prev = c_t
                if s == WIN - 1:
                    nc.gpsimd.dma_start(out_d[w], ob[:])

    return nc


def _get_nc(t_scan):
    key = t_scan
    if key not in _BUILT:
        _BUILT[key] = _build(t_scan)
    return _BUILT[key]


_RUNNERS = {}


def _make_runner(t_scan):
    """Compile once, return a callable in_maps -> list[dict] that only
    executes (PJRT executable cached across kernel() calls)."""
    import jax
    import jax.numpy as jnp
    import numpy as np
    from jax.sharding import Mesh, PartitionSpec
    from jax.experimental.shard_map import shard_map
    from concourse import bass2jax, mybir
    from concourse.bass2jax import _bass_exec_p, install_neuronx_cc_hook

    install_neuronx_cc_hook()
    nc = _get_nc(t_scan)
    assert nc.dbg_addr is None
    n_cores = N_CORES
    partition_name = (nc.partition_id_tensor.name
                      if nc.partition_id_tensor else None)
    in_names, out_names, out_avals, zero_shapes = [], [], [], []
    for alloc in nc.m.functions[0].allocations:
        if not isinstance(alloc, mybir.MemoryLocationSet):
            continue
        name = alloc.memorylocations[0].name
        if alloc.kind == "ExternalInput":
            if name != partition_name:
                in_names.append(name)
        elif alloc.kind == "ExternalOutput":
            shape = tuple(alloc.tensor_shape)
            npdt = mybir.dt.np(alloc.dtype)
            out_avals.append(jax.core.ShapedArray(shape, npdt))
            out_names.append(name)
            zero_shapes.append((shape, npdt))
    n_params = len(in_names)
    n_outs = len(out_names)
    all_in = in_names + out_names
    if partition_name is not None:
        all_in = all_in + [partition_name]

    def _body(*args):
        operands = list(args)
        if partition_name is not None:
            operands.append(bass2jax.partition_id_tensor())
        outs = _bass_exec_p.bind(
            *operands,
            out_avals=tuple(out_avals),
            in_names=tuple(all_in),
            out_names=tuple(out_names),
            lowering_input_output_aliases=(),
            sim_require_finite=True,
            sim_require_nnan=True,
            nc=nc,
        )
        return tuple(outs)

    devices = jax.devices()[:n_cores]
    mesh = Mesh(np.asarray(devices), ("core",))
    donate = tuple(range(n_params, n_params + n_outs))
    sharded = jax.jit(
        shard_map(_body, mesh=mesh,
                  in_specs=(PartitionSpec("core"),) * (n_params + n_outs),
                  out_specs=(PartitionSpec("core"),) * n_outs,
                  check_rep=False),
        donate_argnums=donate, keep_unused=True,
    )

    def run(in_maps):
        concat_in = [
            np.concatenate([np.asarray(m[name]) for m in in_maps], axis=0)
            for name in in_names
        ]
        concat_zeros = [
            jnp.zeros((n_cores * s[0], *s[1:]), dt) for s, dt in zero_shapes
        ]
        out_arrs = sharded(*concat_in, *concat_zeros)
        return [
            {name: np.asarray(out_arrs[i]).reshape(
                n_cores, *out_avals[i].shape)[c]
             for i, name in enumerate(out_names)}
            for c in range(n_cores)
        ]

    run.in_names = in_names
    run.out_names = out_names
    run.zero_shapes = zero_shapes
    run.sharded = sharded
    run.n_cores = n_cores
    return run


def _get_runner(t_scan):
    if t_scan not in _RUNNERS:
        _RUNNERS[t_scan] = _make_runner(t_scan)
    return _RUNNERS[t_scan]


_GATE_PERM = None


def _gate_perm():
    global _GATE_PERM
    if _GATE_PERM is None:
        # reference gate row order i,f,g,o -> kernel order i,f,o,g
        _GATE_PERM = np.concatenate([
            np.arange(0, H), np.arange(H, 2 * H),
            np.arange(3 * H, 4 * H), np.arange(2 * H, 3 * H)])
    return _GATE_PERM


def make_in_maps(x, W_ih_f, W_hh_f, b_f, W_ih_b, W_hh_b, b_b):
    """Host-side input prep: one map per core (0=forward, 1=backward)."""
    x = np.asarray(x, dtype=np.float32)
    perm = _gate_perm()
    in_maps = []
    ident = np.eye(128, dtype=np.float16)
    for d, (wih, whh, bb) in enumerate(
            [(W_ih_f, W_hh_f, b_f), (W_ih_b, W_hh_b, b_b)]):
        wihp = np.asarray(wih, np.float32)[perm].copy()
        whhp = np.asarray(whh, np.float32)[perm].copy()
        bp = np.asarray(bb, np.float32)[perm].copy()
        # pre-scale g-gate rows by 2: tanh(x) = 2*sigmoid(2x) - 1
        wihp[3 * H:] *= 2.0
        whhp[3 * H:] *= 2.0
        bp[3 * H:] *= 2.0
        xd = x if d == 0 else x[:, ::-1]
        xt = np.ascontiguousarray(
            xd.transpose(2, 1, 0).reshape(I, TB)).astype(np.float16)
        in_maps.append({
            "xT": xt,
            "wihT": np.ascontiguousarray(wihp.T).astype(np.float16),
            "whhT": np.ascontiguousarray(whhp.T).astype(np.float16),
            "bsb": np.ascontiguousarray(bp.reshape(M, 128).T),
            "ident": ident,
        })
    return in_maps


def unscramble(results, t_scan):
    """results: list of per-core out dicts -> full [32, t_scan, 1024]."""
    n_win = t_scan // WIN
    halves = []
    for d in range(2):
        raw = np.asarray(results[d]["out_raw"])  # [n_win, 128, WIN*KH*B]
        h = raw.reshape(n_win, 128, WIN, KH, B)
        h = np.ascontiguousarray(h.transpose(4, 0, 2, 3, 1))
        h = h.reshape(B, n_win * WIN, H)[:, :t_scan]
        if d == 1:
            h = h[:, ::-1]
        halves.append(h)
    return np.concatenate(halves, axis=2).astype(np.float32)


def kernel(x, W_ih_f, W_hh_f, b_f, W_ih_b, W_hh_b, b_b, _t_scan=T_SCAN):
    in_maps = make_in_maps(x, W_ih_f, W_hh_f, b_f, W_ih_b, W_hh_b, b_b)
    try:
        runner = _get_runner(_t_scan)
        results = runner(in_maps)
    except Exception:
        from concourse.bass_utils import run_bass_kernel_spmd
        res = run_bass_kernel_spmd(_get_nc(_t_scan), in_maps,
                                   list(range(N_CORES)))
        results = res.results
    return unscramble(results, _t_scan)


# revision 14
# speedup vs baseline: 1.1818x; 1.0364x over previous
"""Bidirectional LSTM on trn2 NeuronCores.

Sharding: 2 cores, one per direction, full batch B=32 per core. The
backward core receives time-reversed x and its output is re-reversed on
the host. The scan is fully core-local (the recurrence never crosses the
wire), and using 2 cores instead of 8 minimizes total device-seconds:
the scan cost is dominated by W_hh stationary-weight ingestion into the
PE array (64 LDWEIGHTS x 128x128 fp16 per step), which is independent of
the per-core batch size, so batch-splitting across more cores multiplies
device time without reducing latency.

Per-core plan (B=32, T=512, I=256, H=512, G=4H=2048):
  1. Host pre-transposes/casts weights and x to fp16 (lhsT / moving
     layouts, t-major x). Gate blocks permuted to i,f,o,g so sigmoid
     covers one contiguous 96-col span per chunk.
  2. Phase C: xp = x @ W_ih.T + b for all T, written to a DRAM buffer in
     16-step window layout (doesn't fit SBUF at B=32), N=512 moving
     columns per matmul so LDWEIGHTS is fully amortized.
  3. 512-step scan, chunk-pipelined: gates are computed per H-chunk k
     (16 matmuls -> PSUM tile [128, 4x32]), and each chunk's
     DVE/ACT tail (add xp, sigmoid/tanh, cell update) runs while the PE
     streams the next chunk's weights. h chunks are written straight
     into the fp16 output window tile, which doubles as next step's
     moving operand, so the serial tail at a step boundary is one chunk
     deep instead of a full step.
  4. xp windows stream DRAM->SBUF double-buffered; output windows
     (16 steps) DMA out as they complete; host unscrambles + upcasts.

The compiled PJRT executable is cached at module level: repeat kernel()
calls only transfer fresh inputs and execute.
"""

import numpy as np

B_FULL, T, I, H = 32, 512, 256, 512
G = 4 * H
N_CORES = 2
B = B_FULL                # per-core batch (one direction per core)
KH = H // 128             # 4 contraction chunks for W_hh
KI = I // 128             # 2 contraction chunks for W_ih
M = G // 128              # 16 gate-row chunks (4 per gate)
WIN = 16                  # scan steps per xp/output window
TB = T * B                # 16384 moving columns, t-major
WCOL = M * WIN * B        # 8192 xp columns per window
T_SCAN = T

_BUILT = {}


def _install_tile_patch():
    """This container's walrus accepts only ONE sync-wait per CTRL-class
    instruction (Drain/NoOp). Tile's kernel-tail drain aggregates one wait
    per semaphore lane onto a single Drain -> split them one per drain."""
    import bass_rust
    import concourse.tile as tile

    if getattr(tile.TileContext, "_drain_split_patched", False):
        return

    def _patched_dab(self, tick_clock, wait_clock):
        from concourse.tile import ScopedClock

        nc = self.nc
        drain_inst = nc.sync.drain()
        wait_clock.add_sem_waits(
            drain_inst.ins, ScopedClock({None: tick_clock.global_clock})
        )
        si = drain_inst.ins.sync_info
        waits = list(si.on_wait) if si is not None else []
        if len(waits) > 1:
            si.on_wait = waits[:1]
            for w in waits[1:]:
                d2 = nc.sync.drain()
                si2 = d2.ins.sync_info
                if si2 is None:
                    d2.ins.sync_info = bass_rust.SyncInfo(on_wait=[w], on_update=[])
                else:
                    si2.on_wait = list(si2.on_wait) + [w]
        nc.all_engine_barrier()
        assert self.sems is not None
        popped = nc._tile_sem_poison_stack.pop()
        assert popped is self._sem_poison
        nc.clear_and_free_semaphores(list(self.sems.allocated().values()))
        nc.all_engine_barrier()

    tile.TileContext._drain_and_barrier = _patched_dab
    tile.TileContext._drain_split_patched = True

    # This walrus build accepts at most ONE sync-wait per instruction (any
    # opcode). Split every multi-wait instruction at BIR-JSON level into
    # single-wait NoOps followed by the real instruction with one wait.
    import json
    import concourse.bass as bass

    if getattr(bass.Bass, "_json_wait_split_patched", False):
        return
    _orig_tjb = bass.Bass.to_json_bytes

    def _split_json(self):
        raw = _orig_tjb(self)
        m = json.loads(raw)
        ctr = 0
        changed = False
        for fn in m.get("functions", []):
            for bb in fn.get("blocks", []):
                out = []
                for inst in bb.get("instructions", []):
                    si = inst.get("sync_info")
                    waits = (si or {}).get("on_wait") or []
                    if len(waits) > 1:
                        changed = True
                        for w in waits[:-1]:
                            ctr += 1
                            nop = {
                                "engine": inst["engine"],
                                "ins": [],
                                "outs": [],
                                "name": f"WSPLIT-{ctr}",
                                "opcode": "NoOp",
                                "sync_info": {"on_update": [], "on_wait": [w]},
                            }
                            if "debug" in inst:
                                nop["debug"] = inst["debug"]
                            out.append(nop)
                        si["on_wait"] = [waits[-1]]
                    out.append(inst)
                bb["instructions"] = out
        if not changed:
            return raw
        return json.dumps(m).encode()

    bass.Bass.to_json_bytes = _split_json
    bass.Bass._json_wait_split_patched = True


def _build(t_scan):
    import concourse.bass as bass
    import concourse.tile as tile
    from concourse import mybir
    from contextlib import ExitStack

    _install_tile_patch()
    f32 = mybir.dt.float32
    f16 = mybir.dt.float16

    assert t_scan % WIN == 0
    n_win = t_scan // WIN

    nc = bass.Bass()
    # Host layouts: xT [I, T*B] f16 t-major (col t*B + b), wihT [I, G] f16,
    # whhT [H, G] f16 (G rows permuted to gate order i,f,o,g; g-gate rows
    # pre-scaled by 2 so tanh(x) = 2*sigmoid(2x)-1 folds into the single
    # sigmoid pass), bsb [128, M], ident = eye(128) f16.
    xt_d = nc.dram_tensor("xT", [I, TB], f16, kind="ExternalInput")
    wiht_d = nc.dram_tensor("wihT", [I, G], f16, kind="ExternalInput")
    whht_d = nc.dram_tensor("whhT", [H, G], f16, kind="ExternalInput")
    bsb_d = nc.dram_tensor("bsb", [128, M], f32, kind="ExternalInput")
    id_d = nc.dram_tensor("ident", [128, 128], f16, kind="ExternalInput")
    # out[w, p, s*128 + k*32 + b] = h[b, 16w+s, 128k+p]
    out_d = nc.dram_tensor("out_raw", [n_win, 128, WIN * KH * B], f16,
                           kind="ExternalOutput")

    with tile.TileContext(nc) as tc, ExitStack() as ctx:
        sig = mybir.ActivationFunctionType.Sigmoid
        tanh = mybir.ActivationFunctionType.Tanh

        wpool = ctx.enter_context(tc.tile_pool(name="w", bufs=1))
        dpool = ctx.enter_context(tc.tile_pool(name="d", bufs=1, space="DRAM"))
        whhT = wpool.tile([128, KH * G], f16)    # tile (kk,m) at (kk*M+m)*128
        wihT = wpool.tile([128, KI * G], f16)
        xT = wpool.tile([128, KI * TB], f16)     # chunk ki at ki*TB
        b_sb = wpool.tile([128, M], f32)
        ident = wpool.tile([128, 128], f16)
        # xp DRAM buffer: one tile PER WINDOW so the scan's window-w read
        # depends only on window-w phase-C writes (tile-granular deps),
        # letting phase C interleave with the scan instead of serializing.
        # window layout: col m*(WIN*B) + s*B + b
        xp_dram = [dpool.tile([128, WCOL], f16, tag=f"xpw{w}",
                              name=f"xp_dram_w{w}")
                   for w in range(n_win)]
        nc.gpsimd.dma_start(b_sb[:], bsb_d[:])
        nc.gpsimd.dma_start(ident[:], id_d[:])
        for k in range(KH):
            nc.gpsimd.dma_start(whhT[:, k * G:(k + 1) * G],
                                whht_d[k * 128:(k + 1) * 128, :])
        for k in range(KI):
            nc.gpsimd.dma_start(wihT[:, k * G:(k + 1) * G],
                                wiht_d[k * 128:(k + 1) * 128, :])
            nc.gpsimd.dma_start(xT[:, k * TB:(k + 1) * TB],
                                xt_d[k * 128:(k + 1) * 128, :])

        # ---- phase C (interleaved with the scan below):
        #      xp = x @ W_ih.T + b -> DRAM window tiles, fp16 ----
        NXP = WIN * B  # 512 moving columns = one window of one m-chunk
        xpp = ctx.enter_context(tc.tile_pool(name="xpps", bufs=2, space="PSUM"))
        xsb = ctx.enter_context(tc.tile_pool(name="xpsb", bufs=3))

        def phase_c_unit(w, mq):
            sb = xsb.tile([128, 4 * NXP], f16, tag="xsb")
            for mi in range(4):
                m = mq * 4 + mi
                ps = xpp.tile([128, NXP], f32, tag="xps")
                for k in range(KI):
                    nc.tensor.matmul(
                        ps[:],
                        wihT[:, (k * M + m) * 128:(k * M + m + 1) * 128],
                        xT[:, k * TB + w * NXP:k * TB + (w + 1) * NXP],
                        start=(k == 0), stop=(k == KI - 1),
                    )
                dst = sb[:, mi * NXP:(mi + 1) * NXP]
                if m % 2 == 0:
                    nc.vector.tensor_scalar_add(dst, ps[:], b_sb[:, m:m + 1])
                else:
                    nc.scalar.add(dst, ps[:], b_sb[:, m:m + 1])
            nc.gpsimd.dma_start(
                xp_dram[w][:, mq * 4 * NXP:(mq + 1) * 4 * NXP], sb[:])

        # prologue: windows 0 and 1; the rest interleave into the scan
        for w in range(min(2, n_win)):
            for mq in range(M // 4):
                phase_c_unit(w, mq)

        # ---- phase D: the scan ----
        # gate m-chunk = g*4 + k (g in i,f,o,g order; k = H 128-chunk)
        # h/c col layout: k*32 + b
        with tc.tile_pool(name="gp", bufs=6, space="PSUM") as gp, \
             tc.tile_pool(name="xpw", bufs=2) as xpool, \
             tc.tile_pool(name="acts", bufs=4) as ap, \
             tc.tile_pool(name="state", bufs=2) as stp, \
             tc.tile_pool(name="outb", bufs=2) as obp, \
             tc.tile_pool(name="init", bufs=1) as ip:
            h0 = ip.tile([128, KH * B], f16)
            c0 = ip.tile([128, KH * B], f32)
            nc.vector.memset(h0[:], 0.0)
            nc.vector.memset(c0[:], 0.0)

            def load_window(w):
                tl = xpool.tile([128, WCOL], f16, tag="xp")
                nc.gpsimd.dma_start(tl[:], xp_dram[w][:])
                return tl

            xpw_cur = load_window(0)
            xpw_next = None
            h_src = h0
            c_prev = c0
            ob = None
            for t in range(t_scan):
                w, s = divmod(t, WIN)
                if s == 0:
                    if w > 0:
                        xpw_cur = xpw_next
                    if w + 1 < n_win:
                        xpw_next = load_window(w + 1)
                    ob = obp.tile([128, WIN * KH * B], f16, tag="ob")
                if s % 4 == 2 and w + 2 < n_win:
                    # phase C for window w+2, one quarter per 4 steps
                    phase_c_unit(w + 2, s // 4)
                # xp view: [p, g, k, s, b]
                xp5 = xpw_cur.rearrange("p (g k s b) -> p g k s b",
                                        g=4, k=KH, s=WIN)
                c_t = stp.tile([128, KH * B], f32, tag="c")
                # all 4 xp preloads first: ready PE work at the step
                # boundary while h chunks of the previous step finish
                pss = []
                for k in range(KH):
                    ps = gp.tile([128, 4 * B], f32, tag="ps")  # i|f|o|g~
                    nc.tensor.matmul(ps[:], ident[:], xp5[:, :, k, s, :],
                                     start=True, stop=False)
                    pss.append(ps)
                # af_all[:, k*128 : (k+1)*128] = sigmoid(gates chunk k)
                af_all = ap.tile([128, KH * 4 * B], f32, tag="af")
                af4 = af_all.rearrange("p (k2 g b) -> p k2 g b", k2=KH, g=4)
                for k in range(KH):
                    ps = pss[k]
                    for g in range(4):
                        m = g * KH + k
                        for kk in range(KH):
                            nc.tensor.matmul(
                                ps[:, g * B:(g + 1) * B],
                                whhT[:, (kk * M + m) * 128:(kk * M + m + 1) * 128],
                                h_src[:, kk * B:(kk + 1) * B],
                                start=False, stop=(kk == KH - 1),
                            )
                    # tail for chunk k: af = sigmoid over all 4 blocks;
                    # g~ = sigmoid(2*g_pre) (host pre-scaled), so
                    # i*g = 2*(g~ - 0.5)*i and c = 2*q + f*c_prev.
                    af = af_all[:, k * 4 * B:(k + 1) * 4 * B]
                    nc.scalar.activation(af, ps[:], sig)
                    q = ap.tile([128, B], f32, tag="q")
                    nc.vector.scalar_tensor_tensor(
                        q[:], af[:, 3 * B:4 * B], 0.5, af[:, 0:B],
                        op0=mybir.AluOpType.subtract, op1=mybir.AluOpType.mult)
                    fc = ap.tile([128, B], f32, tag="fc")
                    nc.gpsimd.tensor_mul(fc[:], af[:, B:2 * B],
                                         c_prev[:, k * B:(k + 1) * B])
                    nc.vector.scalar_tensor_tensor(
                        c_t[:, k * B:(k + 1) * B], q[:], 2.0, fc[:],
                        op0=mybir.AluOpType.mult, op1=mybir.AluOpType.add)
                    if k % 2 == 1:
                        # paired tanh(c) + h = o * th over chunks k-1, k
                        th = ap.tile([128, 2 * B], f32, tag="th")
                        nc.scalar.activation(
                            th[:], c_t[:, (k - 1) * B:(k + 1) * B], tanh)
                        ho = ob.rearrange("p (s2 k2 b) -> p s2 k2 b",
                                          s2=WIN, k2=KH)
                        nc.vector.tensor_mul(
                            ho[:, s, k - 1:k + 1, :],
                            th.rearrange("p (k2 b) -> p k2 b", k2=2),
                            af4[:, k - 1:k + 1, 2, :],
                        )
                h_src = ob[:, s * KH * B:(s + 1) * KH * B]
                c_prev = c_t
                if s == WIN - 1:
                    nc.gpsimd.dma_start(out_d[w], ob[:])

    return nc


def _get_nc(t_scan):
    key = t_scan
    if key not in _BUILT:
        _BUILT[key] = _build(t_scan)
    return _BUILT[key]


_RUNNERS = {}


def _make_runner(t_scan):
    """Compile once, return a callable in_maps -> list[dict] that only
    executes (PJRT executable cached across kernel() calls)."""
    import jax
    import jax.numpy as jnp
    import numpy as np
    from jax.sharding import Mesh, PartitionSpec
    from jax.experimental.shard_map import shard_map
    from concourse import bass2jax, mybir
    from concourse.bass2jax import _bass_exec_p, install_neuronx_cc_hook

    install_neuronx_cc_hook()
    nc = _get_nc(t_scan)
    assert nc.dbg_addr is None
    n_cores = N_CORES
    partition_name = (nc.partition_id_tensor.name
                      if nc.partition_id_tensor else None)
    in_names, out_names, out_avals, zero_shapes = [], [], [], []
    for alloc in nc.m.functions[0].allocations:
        if not isinstance(alloc, mybir.MemoryLocationSet):
            continue
        name = alloc.memorylocations[0].name
        if alloc.kind == "ExternalInput":
            if name != partition_name:
                in_names.append(name)
        elif alloc.kind == "ExternalOutput":
            shape = tuple(alloc.tensor_shape)
            npdt = mybir.dt.np(alloc.dtype)
            out_avals.append(jax.core.ShapedArray(shape, npdt))
            out_names.append(name)
            zero_shapes.append((shape, npdt))
    n_params = len(in_names)
    n_outs = len(out_names)
    all_in = in_names + out_names
    if partition_name is not None:
        all_in = all_in + [partition_name]

    def _body(*args):
        operands = list(args)
        if partition_name is not None:
            operands.append(bass2jax.partition_id_tensor())
        outs = _bass_exec_p.bind(
            *operands,
            out_avals=tuple(out_avals),
            in_names=tuple(all_in),
            out_names=tuple(out_names),
            lowering_input_output_aliases=(),
            sim_require_finite=True,
            sim_require_nnan=True,
            nc=nc,
        )
        return tuple(outs)

    devices = jax.devices()[:n_cores]
    mesh = Mesh(np.asarray(devices), ("core",))
    donate = tuple(range(n_params, n_params + n_outs))
    sharded = jax.jit(
        shard_map(_body, mesh=mesh,
                  in_specs=(PartitionSpec("core"),) * (n_params + n_outs),
                  out_specs=(PartitionSpec("core"),) * n_outs,
                  check_rep=False),
        donate_argnums=donate, keep_unused=True,
    )

    def run(in_maps):
        concat_in = [
            np.concatenate([np.asarray(m[name]) for m in in_maps], axis=0)
            for name in in_names
        ]
        concat_zeros = [
            jnp.zeros((n_cores * s[0], *s[1:]), dt) for s, dt in zero_shapes
        ]
        out_arrs = sharded(*concat_in, *concat_zeros)
        return [
            {name: np.asarray(out_arrs[i]).reshape(
                n_cores, *out_avals[i].shape)[c]
             for i, name in enumerate(out_names)}
            for c in range(n_cores)
        ]

    run.in_names = in_names
    run.out_names = out_names
    run.zero_shapes = zero_shapes
    run.sharded = sharded
    run.n_cores = n_cores
    return run


def _get_runner(t_scan):
    if t_scan not in _RUNNERS:
        _RUNNERS[t_scan] = _make_runner(t_scan)
    return _RUNNERS[t_scan]


_GATE_PERM = None


def _gate_perm():
    global _GATE_PERM
    if _GATE_PERM is None:
        # reference gate row order i,f,g,o -> kernel order i,f,o,g
        _GATE_PERM = np.concatenate([
            np.arange(0, H), np.arange(H, 2 * H),
            np.arange(3 * H, 4 * H), np.arange(2 * H, 3 * H)])
    return _GATE_PERM


def make_in_maps(x, W_ih_f, W_hh_f, b_f, W_ih_b, W_hh_b, b_b):
    """Host-side input prep: one map per core (0=forward, 1=backward)."""
    x = np.asarray(x, dtype=np.float32)
    perm = _gate_perm()
    in_maps = []
    ident = np.eye(128, dtype=np.float16)
    for d, (wih, whh, bb) in enumerate(
            [(W_ih_f, W_hh_f, b_f), (W_ih_b, W_hh_b, b_b)]):
        wihp = np.asarray(wih, np.float32)[perm].copy()
        whhp = np.asarray(whh, np.float32)[perm].copy()
        bp = np.asarray(bb, np.float32)[perm].copy()
        # pre-scale g-gate rows by 2: tanh(x) = 2*sigmoid(2x) - 1
        wihp[3 * H:] *= 2.0
        whhp[3 * H:] *= 2.0
        bp[3 * H:] *= 2.0
        xd = x if d == 0 else x[:, ::-1]
        xt = np.ascontiguousarray(
            xd.transpose(2, 1, 0).reshape(I, TB)).astype(np.float16)
        in_maps.append({
            "xT": xt,
            "wihT": np.ascontiguousarray(wihp.T).astype(np.float16),
            "whhT": np.ascontiguousarray(whhp.T).astype(np.float16),
            "bsb": np.ascontiguousarray(bp.reshape(M, 128).T),
            "ident": ident,
        })
    return in_maps


def unscramble(results, t_scan):
    """results: list of per-core out dicts -> full [32, t_scan, 1024]."""
    n_win = t_scan // WIN
    halves = []
    for d in range(2):
        raw = np.asarray(results[d]["out_raw"])  # [n_win, 128, WIN*KH*B]
        h = raw.reshape(n_win, 128, WIN, KH, B)
        h = np.ascontiguousarray(h.transpose(4, 0, 2, 3, 1))
        h = h.reshape(B, n_win * WIN, H)[:, :t_scan]
        if d == 1:
            h = h[:, ::-1]
        halves.append(h)
    return np.concatenate(halves, axis=2).astype(np.float32)


def kernel(x, W_ih_f, W_hh_f, b_f, W_ih_b, W_hh_b, b_b, _t_scan=T_SCAN):
    in_maps = make_in_maps(x, W_ih_f, W_hh_f, b_f, W_ih_b, W_hh_b, b_b)
    try:
        runner = _get_runner(_t_scan)
        results = runner(in_maps)
    except Exception:
        from concourse.bass_utils import run_bass_kernel_spmd
        res = run_bass_kernel_spmd(_get_nc(_t_scan), in_maps,
                                   list(range(N_CORES)))
        results = res.results
    return unscramble(results, _t_scan)


# revision 18
# speedup vs baseline: 1.2127x; 1.0261x over previous
"""Bidirectional LSTM on trn2 NeuronCores.

Sharding: 2 cores, one per direction, full batch B=32 per core. The
backward core receives time-reversed x and its output is re-reversed on
the host. The scan is fully core-local (the recurrence never crosses the
wire), and using 2 cores instead of 8 minimizes total device-seconds:
the scan cost is dominated by W_hh stationary-weight ingestion into the
PE array (64 LDWEIGHTS x 128x128 fp16 per step), which is independent of
the per-core batch size, so batch-splitting across more cores multiplies
device time without reducing latency.

Per-core plan (B=32, T=512, I=256, H=512, G=4H=2048):
  1. Host pre-transposes/casts weights and x to fp16 (lhsT / moving
     layouts, t-major x). Gate blocks permuted to i,f,o,g so sigmoid
     covers one contiguous 96-col span per chunk.
  2. Phase C: xp = x @ W_ih.T + b for all T, written to a DRAM buffer in
     16-step window layout (doesn't fit SBUF at B=32), N=512 moving
     columns per matmul so LDWEIGHTS is fully amortized.
  3. 512-step scan, chunk-pipelined: gates are computed per H-chunk k
     (16 matmuls -> PSUM tile [128, 4x32]), and each chunk's
     DVE/ACT tail (add xp, sigmoid/tanh, cell update) runs while the PE
     streams the next chunk's weights. h chunks are written straight
     into the fp16 output window tile, which doubles as next step's
     moving operand, so the serial tail at a step boundary is one chunk
     deep instead of a full step.
  4. xp windows stream DRAM->SBUF double-buffered; output windows
     (16 steps) DMA out as they complete; host unscrambles + upcasts.

The compiled PJRT executable is cached at module level: repeat kernel()
calls only transfer fresh inputs and execute.
"""

import numpy as np

B_FULL, T, I, H = 32, 512, 256, 512
G = 4 * H
N_CORES = 2
B = B_FULL                # per-core batch (one direction per core)
KH = H // 128             # 4 contraction chunks for W_hh
KI = I // 128             # 2 contraction chunks for W_ih
M = G // 128              # 16 gate-row chunks (4 per gate)
WIN = 16                  # scan steps per xp/output window
TB = T * B                # 16384 moving columns, t-major
WCOL = M * WIN * B        # 8192 xp columns per window
T_SCAN = T

_BUILT = {}


def _install_tile_patch():
    """This container's walrus accepts only ONE sync-wait per CTRL-class
    instruction (Drain/NoOp). Tile's kernel-tail drain aggregates one wait
    per semaphore lane onto a single Drain -> split them one per drain."""
    import bass_rust
    import concourse.tile as tile

    if getattr(tile.TileContext, "_drain_split_patched", False):
        return

    def _patched_dab(self, tick_clock, wait_clock):
        from concourse.tile import ScopedClock

        nc = self.nc
        drain_inst = nc.sync.drain()
        wait_clock.add_sem_waits(
            drain_inst.ins, ScopedClock({None: tick_clock.global_clock})
        )
        si = drain_inst.ins.sync_info
        waits = list(si.on_wait) if si is not None else []
        if len(waits) > 1:
            si.on_wait = waits[:1]
            for w in waits[1:]:
                d2 = nc.sync.drain()
                si2 = d2.ins.sync_info
                if si2 is None:
                    d2.ins.sync_info = bass_rust.SyncInfo(on_wait=[w], on_update=[])
                else:
                    si2.on_wait = list(si2.on_wait) + [w]
        nc.all_engine_barrier()
        assert self.sems is not None
        popped = nc._tile_sem_poison_stack.pop()
        assert popped is self._sem_poison
        nc.clear_and_free_semaphores(list(self.sems.allocated().values()))
        nc.all_engine_barrier()

    tile.TileContext._drain_and_barrier = _patched_dab
    tile.TileContext._drain_split_patched = True

    # This walrus build accepts at most ONE sync-wait per instruction (any
    # opcode). Split every multi-wait instruction at BIR-JSON level into
    # single-wait NoOps followed by the real instruction with one wait.
    import json
    import concourse.bass as bass

    if getattr(bass.Bass, "_json_wait_split_patched", False):
        return
    _orig_tjb = bass.Bass.to_json_bytes

    def _split_json(self):
        raw = _orig_tjb(self)
        m = json.loads(raw)
        ctr = 0
        changed = False
        for fn in m.get("functions", []):
            for bb in fn.get("blocks", []):
                out = []
                for inst in bb.get("instructions", []):
                    si = inst.get("sync_info")
                    waits = (si or {}).get("on_wait") or []
                    if len(waits) > 1:
                        changed = True
                        for w in waits[:-1]:
                            ctr += 1
                            nop = {
                                "engine": inst["engine"],
                                "ins": [],
                                "outs": [],
                                "name": f"WSPLIT-{ctr}",
                                "opcode": "NoOp",
                                "sync_info": {"on_update": [], "on_wait": [w]},
                            }
                            if "debug" in inst:
                                nop["debug"] = inst["debug"]
                            out.append(nop)
                        si["on_wait"] = [waits[-1]]
                    out.append(inst)
                bb["instructions"] = out
        if not changed:
            return raw
        return json.dumps(m).encode()

    bass.Bass.to_json_bytes = _split_json
    bass.Bass._json_wait_split_patched = True


def _build(t_scan):
    import concourse.bass as bass
    import concourse.tile as tile
    from concourse import mybir
    from contextlib import ExitStack

    _install_tile_patch()
    f32 = mybir.dt.float32
    f16 = mybir.dt.float16

    assert t_scan % WIN == 0
    n_win = t_scan // WIN

    nc = bass.Bass()
    # Host layouts: xT [I, T*B] f16 t-major (col t*B + b), wihT [I, G] f16,
    # whhT [H, G] f16 (G rows permuted to gate order i,f,o,g; g-gate rows
    # pre-scaled by 2 so tanh(x) = 2*sigmoid(2x)-1 folds into the single
    # sigmoid pass), bsb [128, M], ident = eye(128) f16.
    xt_d = nc.dram_tensor("xT", [I, TB], f16, kind="ExternalInput")
    wiht_d = nc.dram_tensor("wihT", [I, G], f16, kind="ExternalInput")
    whht_d = nc.dram_tensor("whhT", [H, G], f16, kind="ExternalInput")
    bsb_d = nc.dram_tensor("bsb", [128, M], f32, kind="ExternalInput")
    id_d = nc.dram_tensor("ident", [128, 128], f16, kind="ExternalInput")
    # out[w, p, s*128 + k*32 + b] = h[b, 16w+s, 128k+p]
    out_d = nc.dram_tensor("out_raw", [n_win, 128, WIN * KH * B], f16,
                           kind="ExternalOutput")

    with tile.TileContext(nc) as tc, ExitStack() as ctx:
        sig = mybir.ActivationFunctionType.Sigmoid
        tanh = mybir.ActivationFunctionType.Tanh

        wpool = ctx.enter_context(tc.tile_pool(name="w", bufs=1))
        dpool = ctx.enter_context(tc.tile_pool(name="d", bufs=1, space="DRAM"))
        whhT = wpool.tile([128, KH * G], f16)    # tile (kk,m) at (kk*M+m)*128
        wihT = wpool.tile([128, KI * G], f16)
        xT = wpool.tile([128, KI * TB], f16)     # chunk ki at ki*TB
        b_sb = wpool.tile([128, M], f32)
        ident = wpool.tile([128, 128], f16)
        # xp DRAM buffer: one tile PER WINDOW so the scan's window-w read
        # depends only on window-w phase-C writes (tile-granular deps),
        # letting phase C interleave with the scan instead of serializing.
        # window layout: col m*(WIN*B) + s*B + b
        xp_dram = [dpool.tile([128, WCOL], f16, tag=f"xpw{w}",
                              name=f"xp_dram_w{w}")
                   for w in range(n_win)]
        nc.gpsimd.dma_start(b_sb[:], bsb_d[:])
        nc.gpsimd.dma_start(ident[:], id_d[:])
        for k in range(KH):
            nc.gpsimd.dma_start(whhT[:, k * G:(k + 1) * G],
                                whht_d[k * 128:(k + 1) * 128, :])
        for k in range(KI):
            nc.gpsimd.dma_start(wihT[:, k * G:(k + 1) * G],
                                wiht_d[k * 128:(k + 1) * 128, :])
            nc.gpsimd.dma_start(xT[:, k * TB:(k + 1) * TB],
                                xt_d[k * 128:(k + 1) * 128, :])

        # ---- phase C (interleaved with the scan below):
        #      xp = x @ W_ih.T + b -> DRAM window tiles, fp16 ----
        NXP = WIN * B  # 512 moving columns = one window of one m-chunk
        xpp = ctx.enter_context(tc.tile_pool(name="xpps", bufs=2, space="PSUM"))
        xsb = ctx.enter_context(tc.tile_pool(name="xpsb", bufs=3))

        def phase_c_unit(w, mq):
            sb = xsb.tile([128, 4 * NXP], f16, tag="xsb")
            for mi in range(4):
                m = mq * 4 + mi
                ps = xpp.tile([128, NXP], f32, tag="xps")
                for k in range(KI):
                    nc.tensor.matmul(
                        ps[:],
                        wihT[:, (k * M + m) * 128:(k * M + m + 1) * 128],
                        xT[:, k * TB + w * NXP:k * TB + (w + 1) * NXP],
                        start=(k == 0), stop=(k == KI - 1),
                    )
                dst = sb[:, mi * NXP:(mi + 1) * NXP]
                if m % 2 == 0:
                    nc.vector.tensor_scalar_add(dst, ps[:], b_sb[:, m:m + 1])
                else:
                    nc.scalar.add(dst, ps[:], b_sb[:, m:m + 1])
            nc.gpsimd.dma_start(
                xp_dram[w][:, mq * 4 * NXP:(mq + 1) * 4 * NXP], sb[:])

        # prologue: windows 0 and 1; the rest interleave into the scan,
        # one m-chunk per step (16 m-chunks per 16-step window)
        for w in range(min(2, n_win)):
            for mq in range(M // 4):
                phase_c_unit(w, mq)

        pc_sb = [None]  # staging tile for the in-scan phase-C subunits

        def phase_c_subunit(w, m):
            mi = m % 4
            if mi == 0:
                pc_sb[0] = xsb.tile([128, 4 * NXP], f16, tag="xsb",
                                    name=f"pcsb_{w}_{m}")
            sb = pc_sb[0]
            ps = xpp.tile([128, NXP], f32, tag="xps", name=f"pcps_{w}_{m}")
            for k in range(KI):
                nc.tensor.matmul(
                    ps[:],
                    wihT[:, (k * M + m) * 128:(k * M + m + 1) * 128],
                    xT[:, k * TB + w * NXP:k * TB + (w + 1) * NXP],
                    start=(k == 0), stop=(k == KI - 1),
                )
            dst = sb[:, mi * NXP:(mi + 1) * NXP]
            if m % 2 == 0:
                nc.vector.tensor_scalar_add(dst, ps[:], b_sb[:, m:m + 1])
            else:
                nc.scalar.add(dst, ps[:], b_sb[:, m:m + 1])
            if mi == 3:
                mq = m // 4
                nc.gpsimd.dma_start(
                    xp_dram[w][:, mq * 4 * NXP:(mq + 1) * 4 * NXP], sb[:])

        # ---- phase D: the scan ----
        # gate m-chunk = g*4 + k (g in i,f,o,g order; k = H 128-chunk)
        # h/c col layout: k*32 + b
        with tc.tile_pool(name="gp", bufs=6, space="PSUM") as gp, \
             tc.tile_pool(name="xpw", bufs=2) as xpool, \
             tc.tile_pool(name="acts", bufs=4) as ap, \
             tc.tile_pool(name="state", bufs=2) as stp, \
             tc.tile_pool(name="outb", bufs=2) as obp, \
             tc.tile_pool(name="init", bufs=1) as ip:
            h0 = ip.tile([128, KH * B], f16)
            c0 = ip.tile([128, KH * B], f32)
            nc.vector.memset(h0[:], 0.0)
            nc.vector.memset(c0[:], 0.0)

            def load_window(w):
                tl = xpool.tile([128, WCOL], f16, tag="xp")
                nc.gpsimd.dma_start(tl[:], xp_dram[w][:])
                return tl

            xpw_cur = load_window(0)
            xpw_next = None
            h_src = h0
            c_prev = c0
            ob = None
            for t in range(t_scan):
                w, s = divmod(t, WIN)
                if s == 0:
                    if w > 0:
                        xpw_cur = xpw_next
                    if w + 1 < n_win:
                        xpw_next = load_window(w + 1)
                    ob = obp.tile([128, WIN * KH * B], f16, tag="ob")

                # xp view: [p, g, k, s, b]
                xp5 = xpw_cur.rearrange("p (g k s b) -> p g k s b",
                                        g=4, k=KH, s=WIN)
                c_t = stp.tile([128, KH * B], f32, tag="c")
                # all 4 xp preloads first: ready PE work at the step
                # boundary while h chunks of the previous step finish
                pss = []
                for k in range(KH):
                    ps = gp.tile([128, 4 * B], f32, tag="ps")  # i|f|o|g~
                    nc.tensor.matmul(ps[:], ident[:], xp5[:, :, k, s, :],
                                     start=True, stop=False)
                    pss.append(ps)
                # af_all[:, k*128 : (k+1)*128] = sigmoid(gates chunk k)
                af_all = ap.tile([128, KH * 4 * B], f32, tag="af")
                af4 = af_all.rearrange("p (k2 g b) -> p k2 g b", k2=KH, g=4)
                for k in range(KH):
                    ps = pss[k]
                    # kk OUTER: the chunk's last-needed h arrives at its
                    # 13th matmul, not its 2nd -> shorter boundary stall
                    for kk in range(KH):
                        for g in range(4):
                            m = g * KH + k
                            nc.tensor.matmul(
                                ps[:, g * B:(g + 1) * B],
                                whhT[:, (kk * M + m) * 128:(kk * M + m + 1) * 128],
                                h_src[:, kk * B:(kk + 1) * B],
                                start=False, stop=(kk == KH - 1),
                            )
                    # tail for chunk k: af = sigmoid over all 4 blocks;
                    # g~ = sigmoid(2*g_pre) (host pre-scaled), so
                    # i*g = 2*(g~ - 0.5)*i and c = 2*q + f*c_prev.
                    af = af_all[:, k * 4 * B:(k + 1) * 4 * B]
                    nc.scalar.activation(af, ps[:], sig)
                    q = ap.tile([128, B], f32, tag="q")
                    nc.vector.scalar_tensor_tensor(
                        q[:], af[:, 3 * B:4 * B], 0.5, af[:, 0:B],
                        op0=mybir.AluOpType.subtract, op1=mybir.AluOpType.mult)
                    fc = ap.tile([128, B], f32, tag="fc")
                    nc.gpsimd.tensor_mul(fc[:], af[:, B:2 * B],
                                         c_prev[:, k * B:(k + 1) * B])
                    nc.vector.scalar_tensor_tensor(
                        c_t[:, k * B:(k + 1) * B], q[:], 2.0, fc[:],
                        op0=mybir.AluOpType.mult, op1=mybir.AluOpType.add)
                    if k % 2 == 1:
                        # paired tanh(c) + h = o * th over chunks k-1, k
                        th = ap.tile([128, 2 * B], f32, tag="th")
                        nc.scalar.activation(
                            th[:], c_t[:, (k - 1) * B:(k + 1) * B], tanh)
                        ho = ob.rearrange("p (s2 k2 b) -> p s2 k2 b",
                                          s2=WIN, k2=KH)
                        nc.vector.tensor_mul(
                            ho[:, s, k - 1:k + 1, :],
                            th.rearrange("p (k2 b) -> p k2 b", k2=2),
                            af4[:, k - 1:k + 1, 2, :],
                        )
                    if k == 0 and w + 2 < n_win:
                        # PE filler while the previous step's late h
                        # chunks finish: phase C for window w+2
                        phase_c_subunit(w + 2, s)
                h_src = ob[:, s * KH * B:(s + 1) * KH * B]
                c_prev = c_t
                if s == WIN - 1:
                    nc.gpsimd.dma_start(out_d[w], ob[:])

    return nc


def _get_nc(t_scan):
    key = t_scan
    if key not in _BUILT:
        _BUILT[key] = _build(t_scan)
    return _BUILT[key]


_RUNNERS = {}


def _make_runner(t_scan):
    """Compile once, return a callable in_maps -> list[dict] that only
    executes (PJRT executable cached across kernel() calls)."""
    import jax
    import jax.numpy as jnp
    import numpy as np
    from jax.sharding import Mesh, PartitionSpec
    from jax.experimental.shard_map import shard_map
    from concourse import bass2jax, mybir
    from concourse.bass2jax import _bass_exec_p, install_neuronx_cc_hook

    install_neuronx_cc_hook()
    nc = _get_nc(t_scan)
    assert nc.dbg_addr is None
    n_cores = N_CORES
    partition_name = (nc.partition_id_tensor.name
                      if nc.partition_id_tensor else None)
    in_names, out_names, out_avals, zero_shapes = [], [], [], []
    for alloc in nc.m.functions[0].allocations:
        if not isinstance(alloc, mybir.MemoryLocationSet):
            continue
        name = alloc.memorylocations[0].name
        if alloc.kind == "ExternalInput":
            if name != partition_name:
                in_names.append(name)
        elif alloc.kind == "ExternalOutput":
            shape = tuple(alloc.tensor_shape)
            npdt = mybir.dt.np(alloc.dtype)
            out_avals.append(jax.core.ShapedArray(shape, npdt))
            out_names.append(name)
            zero_shapes.append((shape, npdt))
    n_params = len(in_names)
    n_outs = len(out_names)
    all_in = in_names + out_names
    if partition_name is not None:
        all_in = all_in + [partition_name]

    def _body(*args):
        operands = list(args)
        if partition_name is not None:
            operands.append(bass2jax.partition_id_tensor())
        outs = _bass_exec_p.bind(
            *operands,
            out_avals=tuple(out_avals),
            in_names=tuple(all_in),
            out_names=tuple(out_names),
            lowering_input_output_aliases=(),
            sim_require_finite=True,
            sim_require_nnan=True,
            nc=nc,
        )
        return tuple(outs)

    devices = jax.devices()[:n_cores]
    mesh = Mesh(np.asarray(devices), ("core",))
    donate = tuple(range(n_params, n_params + n_outs))
    sharded = jax.jit(
        shard_map(_body, mesh=mesh,
                  in_specs=(PartitionSpec("core"),) * (n_params + n_outs),
                  out_specs=(PartitionSpec("core"),) * n_outs,
                  check_rep=False),
        donate_argnums=donate, keep_unused=True,
    )

    def run(in_maps):
        concat_in = [
            np.concatenate([np.asarray(m[name]) for m in in_maps], axis=0)
            for name in in_names
        ]
        concat_zeros = [
            jnp.zeros((n_cores * s[0], *s[1:]), dt) for s, dt in zero_shapes
        ]
        out_arrs = sharded(*concat_in, *concat_zeros)
        return [
            {name: np.asarray(out_arrs[i]).reshape(
                n_cores, *out_avals[i].shape)[c]
             for i, name in enumerate(out_names)}
            for c in range(n_cores)
        ]

    run.in_names = in_names
    run.out_names = out_names
    run.zero_shapes = zero_shapes
    run.sharded = sharded
    run.n_cores = n_cores
    return run


def _get_runner(t_scan):
    if t_scan not in _RUNNERS:
        _RUNNERS[t_scan] = _make_runner(t_scan)
    return _RUNNERS[t_scan]


_GATE_PERM = None


def _gate_perm():
    global _GATE_PERM
    if _GATE_PERM is None:
        # reference gate row order i,f,g,o -> kernel order i,f,o,g
        _GATE_PERM = np.concatenate([
            np.arange(0, H), np.arange(H, 2 * H),
            np.arange(3 * H, 4 * H), np.arange(2 * H, 3 * H)])
    return _GATE_PERM


def make_in_maps(x, W_ih_f, W_hh_f, b_f, W_ih_b, W_hh_b, b_b):
    """Host-side input prep: one map per core (0=forward, 1=backward)."""
    x = np.asarray(x, dtype=np.float32)
    perm = _gate_perm()
    in_maps = []
    ident = np.eye(128, dtype=np.float16)
    for d, (wih, whh, bb) in enumerate(
            [(W_ih_f, W_hh_f, b_f), (W_ih_b, W_hh_b, b_b)]):
        wihp = np.asarray(wih, np.float32)[perm].copy()
        whhp = np.asarray(whh, np.float32)[perm].copy()
        bp = np.asarray(bb, np.float32)[perm].copy()
        # pre-scale g-gate rows by 2: tanh(x) = 2*sigmoid(2x) - 1
        wihp[3 * H:] *= 2.0
        whhp[3 * H:] *= 2.0
        bp[3 * H:] *= 2.0
        xd = x if d == 0 else x[:, ::-1]
        xt = np.ascontiguousarray(
            xd.transpose(2, 1, 0).reshape(I, TB)).astype(np.float16)
        in_maps.append({
            "xT": xt,
            "wihT": np.ascontiguousarray(wihp.T).astype(np.float16),
            "whhT": np.ascontiguousarray(whhp.T).astype(np.float16),
            "bsb": np.ascontiguousarray(bp.reshape(M, 128).T),
            "ident": ident,
        })
    return in_maps


def unscramble(results, t_scan):
    """results: list of per-core out dicts -> full [32, t_scan, 1024]."""
    n_win = t_scan // WIN
    halves = []
    for d in range(2):
        raw = np.asarray(results[d]["out_raw"])  # [n_win, 128, WIN*KH*B]
        h = raw.reshape(n_win, 128, WIN, KH, B)
        h = np.ascontiguousarray(h.transpose(4, 0, 2, 3, 1))
        h = h.reshape(B, n_win * WIN, H)[:, :t_scan]
        if d == 1:
            h = h[:, ::-1]
        halves.append(h)
    return np.concatenate(halves, axis=2).astype(np.float32)


def kernel(x, W_ih_f, W_hh_f, b_f, W_ih_b, W_hh_b, b_b, _t_scan=T_SCAN):
    in_maps = make_in_maps(x, W_ih_f, W_hh_f, b_f, W_ih_b, W_hh_b, b_b)
    try:
        runner = _get_runner(_t_scan)
        results = runner(in_maps)
    except Exception:
        from concourse.bass_utils import run_bass_kernel_spmd
        res = run_bass_kernel_spmd(_get_nc(_t_scan), in_maps,
                                   list(range(N_CORES)))
        results = res.results
    return unscramble(results, _t_scan)


# revision 19
# speedup vs baseline: 1.2136x; 1.0008x over previous
"""Bidirectional LSTM on trn2 NeuronCores.

Sharding: 2 cores, one per direction, full batch B=32 per core. The
backward core receives time-reversed x and its output is re-reversed on
the host. The scan is fully core-local (the recurrence never crosses the
wire), and using 2 cores instead of 8 minimizes total device-seconds:
the scan cost is dominated by W_hh stationary-weight ingestion into the
PE array (64 LDWEIGHTS x 128x128 fp16 per step), which is independent of
the per-core batch size, so batch-splitting across more cores multiplies
device time without reducing latency.

Per-core plan (B=32, T=512, I=256, H=512, G=4H=2048):
  1. Host pre-transposes/casts weights and x to fp16 (lhsT / moving
     layouts, t-major x). Gate blocks permuted to i,f,o,g so sigmoid
     covers one contiguous 96-col span per chunk.
  2. Phase C: xp = x @ W_ih.T + b for all T, written to a DRAM buffer in
     16-step window layout (doesn't fit SBUF at B=32), N=512 moving
     columns per matmul so LDWEIGHTS is fully amortized.
  3. 512-step scan, chunk-pipelined: gates are computed per H-chunk k
     (16 matmuls -> PSUM tile [128, 4x32]), and each chunk's
     DVE/ACT tail (add xp, sigmoid/tanh, cell update) runs while the PE
     streams the next chunk's weights. h chunks are written straight
     into the fp16 output window tile, which doubles as next step's
     moving operand, so the serial tail at a step boundary is one chunk
     deep instead of a full step.
  4. xp windows stream DRAM->SBUF double-buffered; output windows
     (16 steps) DMA out as they complete; host unscrambles + upcasts.

The compiled PJRT executable is cached at module level: repeat kernel()
calls only transfer fresh inputs and execute.
"""

import numpy as np

B_FULL, T, I, H = 32, 512, 256, 512
G = 4 * H
N_CORES = 2
B = B_FULL                # per-core batch (one direction per core)
KH = H // 128             # 4 contraction chunks for W_hh
KI = I // 128             # 2 contraction chunks for W_ih
M = G // 128              # 16 gate-row chunks (4 per gate)
WIN = 16                  # scan steps per xp/output window
TB = T * B                # 16384 moving columns, t-major
WCOL = M * WIN * B        # 8192 xp columns per window
T_SCAN = T

_BUILT = {}


def _install_tile_patch():
    """This container's walrus accepts only ONE sync-wait per CTRL-class
    instruction (Drain/NoOp). Tile's kernel-tail drain aggregates one wait
    per semaphore lane onto a single Drain -> split them one per drain."""
    import bass_rust
    import concourse.tile as tile

    if getattr(tile.TileContext, "_drain_split_patched", False):
        return

    def _patched_dab(self, tick_clock, wait_clock):
        from concourse.tile import ScopedClock

        nc = self.nc
        drain_inst = nc.sync.drain()
        wait_clock.add_sem_waits(
            drain_inst.ins, ScopedClock({None: tick_clock.global_clock})
        )
        si = drain_inst.ins.sync_info
        waits = list(si.on_wait) if si is not None else []
        if len(waits) > 1:
            si.on_wait = waits[:1]
            for w in waits[1:]:
                d2 = nc.sync.drain()
                si2 = d2.ins.sync_info
                if si2 is None:
                    d2.ins.sync_info = bass_rust.SyncInfo(on_wait=[w], on_update=[])
                else:
                    si2.on_wait = list(si2.on_wait) + [w]
        nc.all_engine_barrier()
        assert self.sems is not None
        popped = nc._tile_sem_poison_stack.pop()
        assert popped is self._sem_poison
        nc.clear_and_free_semaphores(list(self.sems.allocated().values()))
        nc.all_engine_barrier()

    tile.TileContext._drain_and_barrier = _patched_dab
    tile.TileContext._drain_split_patched = True

    # This walrus build accepts at most ONE sync-wait per instruction (any
    # opcode). Split every multi-wait instruction at BIR-JSON level into
    # single-wait NoOps followed by the real instruction with one wait.
    import json
    import concourse.bass as bass

    if getattr(bass.Bass, "_json_wait_split_patched", False):
        return
    _orig_tjb = bass.Bass.to_json_bytes

    def _split_json(self):
        raw = _orig_tjb(self)
        m = json.loads(raw)
        ctr = 0
        changed = False
        for fn in m.get("functions", []):
            for bb in fn.get("blocks", []):
                out = []
                for inst in bb.get("instructions", []):
                    si = inst.get("sync_info")
                    waits = (si or {}).get("on_wait") or []
                    if len(waits) > 1:
                        changed = True
                        for w in waits[:-1]:
                            ctr += 1
                            nop = {
                                "engine": inst["engine"],
                                "ins": [],
                                "outs": [],
                                "name": f"WSPLIT-{ctr}",
                                "opcode": "NoOp",
                                "sync_info": {"on_update": [], "on_wait": [w]},
                            }
                            if "debug" in inst:
                                nop["debug"] = inst["debug"]
                            out.append(nop)
                        si["on_wait"] = [waits[-1]]
                    out.append(inst)
                bb["instructions"] = out
        if not changed:
            return raw
        return json.dumps(m).encode()

    bass.Bass.to_json_bytes = _split_json
    bass.Bass._json_wait_split_patched = True


def _build(t_scan):
    import concourse.bass as bass
    import concourse.tile as tile
    from concourse import mybir
    from contextlib import ExitStack

    _install_tile_patch()
    f32 = mybir.dt.float32
    f16 = mybir.dt.float16

    assert t_scan % WIN == 0
    n_win = t_scan // WIN

    nc = bass.Bass()
    # Host layouts: xT [I, T*B] f16 t-major (col t*B + b), wihT [I, G] f16,
    # whhT [H, G] f16 (G rows permuted to gate order i,f,o,g; g-gate rows
    # pre-scaled by 2 so tanh(x) = 2*sigmoid(2x)-1 folds into the single
    # sigmoid pass), bsb [128, M], ident = eye(128) f16.
    xt_d = nc.dram_tensor("xT", [I, TB], f16, kind="ExternalInput")
    wiht_d = nc.dram_tensor("wihT", [I, G], f16, kind="ExternalInput")
    whht_d = nc.dram_tensor("whhT", [H, G], f16, kind="ExternalInput")
    bsb_d = nc.dram_tensor("bsb", [128, M], f32, kind="ExternalInput")
    id_d = nc.dram_tensor("ident", [128, 128], f16, kind="ExternalInput")
    # out[w, p, s*128 + k*32 + b] = h[b, 16w+s, 128k+p]
    out_d = nc.dram_tensor("out_raw", [n_win, 128, WIN * KH * B], f16,
                           kind="ExternalOutput")

    with tile.TileContext(nc) as tc, ExitStack() as ctx:
        sig = mybir.ActivationFunctionType.Sigmoid
        tanh = mybir.ActivationFunctionType.Tanh

        wpool = ctx.enter_context(tc.tile_pool(name="w", bufs=1))
        dpool = ctx.enter_context(tc.tile_pool(name="d", bufs=1, space="DRAM"))
        whhT = wpool.tile([128, KH * G], f16)    # tile (kk,m) at (kk*M+m)*128
        wihT = wpool.tile([128, KI * G], f16)
        xT = wpool.tile([128, KI * TB], f16)     # chunk ki at ki*TB
        b_sb = wpool.tile([128, M], f32)
        ident = wpool.tile([128, 128], f16)
        # xp DRAM buffer: one tile PER WINDOW so the scan's window-w read
        # depends only on window-w phase-C writes (tile-granular deps),
        # letting phase C interleave with the scan instead of serializing.
        # window layout: col m*(WIN*B) + s*B + b
        xp_dram = [dpool.tile([128, WCOL], f16, tag=f"xpw{w}",
                              name=f"xp_dram_w{w}")
                   for w in range(n_win)]
        nc.gpsimd.dma_start(b_sb[:], bsb_d[:])
        nc.gpsimd.dma_start(ident[:], id_d[:])
        for k in range(KH):
            nc.gpsimd.dma_start(whhT[:, k * G:(k + 1) * G],
                                whht_d[k * 128:(k + 1) * 128, :])
        for k in range(KI):
            nc.gpsimd.dma_start(wihT[:, k * G:(k + 1) * G],
                                wiht_d[k * 128:(k + 1) * 128, :])
            nc.gpsimd.dma_start(xT[:, k * TB:(k + 1) * TB],
                                xt_d[k * 128:(k + 1) * 128, :])

        # ---- phase C (interleaved with the scan below):
        #      xp = x @ W_ih.T + b -> DRAM window tiles, fp16 ----
        NXP = WIN * B  # 512 moving columns = one window of one m-chunk
        xpp = ctx.enter_context(tc.tile_pool(name="xpps", bufs=2, space="PSUM"))
        xsb = ctx.enter_context(tc.tile_pool(name="xpsb", bufs=3))

        def phase_c_unit(w, mq):
            sb = xsb.tile([128, 4 * NXP], f16, tag="xsb")
            for mi in range(4):
                m = mq * 4 + mi
                ps = xpp.tile([128, NXP], f32, tag="xps")
                for k in range(KI):
                    nc.tensor.matmul(
                        ps[:],
                        wihT[:, (k * M + m) * 128:(k * M + m + 1) * 128],
                        xT[:, k * TB + w * NXP:k * TB + (w + 1) * NXP],
                        start=(k == 0), stop=(k == KI - 1),
                    )
                dst = sb[:, mi * NXP:(mi + 1) * NXP]
                if m % 2 == 0:
                    nc.vector.tensor_scalar_add(dst, ps[:], b_sb[:, m:m + 1])
                else:
                    nc.scalar.add(dst, ps[:], b_sb[:, m:m + 1])
            nc.gpsimd.dma_start(
                xp_dram[w][:, mq * 4 * NXP:(mq + 1) * 4 * NXP], sb[:])

        # prologue: windows 0 and 1; the rest interleave into the scan,
        # one m-chunk per step (16 m-chunks per 16-step window)
        for w in range(min(2, n_win)):
            for mq in range(M // 4):
                phase_c_unit(w, mq)

        pc_sb = [None]  # staging tile for the in-scan phase-C subunits

        def phase_c_subunit(w, m):
            mi = m % 4
            if mi == 0:
                pc_sb[0] = xsb.tile([128, 4 * NXP], f16, tag="xsb",
                                    name=f"pcsb_{w}_{m}")
            sb = pc_sb[0]
            ps = xpp.tile([128, NXP], f32, tag="xps", name=f"pcps_{w}_{m}")
            for k in range(KI):
                nc.tensor.matmul(
                    ps[:],
                    wihT[:, (k * M + m) * 128:(k * M + m + 1) * 128],
                    xT[:, k * TB + w * NXP:k * TB + (w + 1) * NXP],
                    start=(k == 0), stop=(k == KI - 1),
                )
            dst = sb[:, mi * NXP:(mi + 1) * NXP]
            if m % 2 == 0:
                nc.vector.tensor_scalar_add(dst, ps[:], b_sb[:, m:m + 1])
            else:
                nc.scalar.add(dst, ps[:], b_sb[:, m:m + 1])
            if mi == 3:
                mq = m // 4
                nc.gpsimd.dma_start(
                    xp_dram[w][:, mq * 4 * NXP:(mq + 1) * 4 * NXP], sb[:])

        # ---- phase D: the scan ----
        # gate m-chunk = g*4 + k (g in i,f,o,g order; k = H 128-chunk)
        # h/c col layout: k*32 + b
        with tc.tile_pool(name="gp", bufs=6, space="PSUM") as gp, \
             tc.tile_pool(name="xpw", bufs=2) as xpool, \
             tc.tile_pool(name="acts", bufs=4) as ap, \
             tc.tile_pool(name="state", bufs=2) as stp, \
             tc.tile_pool(name="outb", bufs=2) as obp, \
             tc.tile_pool(name="init", bufs=1) as ip:
            h0 = ip.tile([128, KH * B], f16)
            c0 = ip.tile([128, KH * B], f32)
            nc.vector.memset(h0[:], 0.0)
            nc.vector.memset(c0[:], 0.0)

            def load_window(w):
                tl = xpool.tile([128, WCOL], f16, tag="xp")
                nc.gpsimd.dma_start(tl[:], xp_dram[w][:])
                return tl

            xpw_cur = load_window(0)
            xpw_next = None
            h_src = h0
            c_prev = c0
            ob = None
            for t in range(t_scan):
                w, s = divmod(t, WIN)
                if s == 0:
                    if w > 0:
                        xpw_cur = xpw_next
                    if w + 1 < n_win:
                        xpw_next = load_window(w + 1)
                    ob = obp.tile([128, WIN * KH * B], f16, tag="ob")

                # xp view: [p, g, k, s, b]
                xp5 = xpw_cur.rearrange("p (g k s b) -> p g k s b",
                                        g=4, k=KH, s=WIN)
                c_t = stp.tile([128, KH * B], f32, tag="c")
                # all 4 xp preloads first: ready PE work at the step
                # boundary while h chunks of the previous step finish
                pss = []
                for k in range(KH):
                    # pad to a full PSUM bank: a shared bank would make the
                    # bank-overlap tracker serialize each chunk's sigmoid
                    # against ALL chunks' matmuls, killing tail pipelining
                    ps = gp.tile([128, 4 * B], f32, tag="ps",
                                 padded_shape=[128, 512])  # i|f|o|g~
                    nc.tensor.matmul(ps[:], ident[:], xp5[:, :, k, s, :],
                                     start=True, stop=False)
                    pss.append(ps)
                # af_all[:, k*128 : (k+1)*128] = sigmoid(gates chunk k)
                af_all = ap.tile([128, KH * 4 * B], f32, tag="af")
                af4 = af_all.rearrange("p (k2 g b) -> p k2 g b", k2=KH, g=4)
                for k in range(KH):
                    ps = pss[k]
                    # kk OUTER: the chunk's last-needed h arrives at its
                    # 13th matmul, not its 2nd -> shorter boundary stall
                    for kk in range(KH):
                        for g in range(4):
                            m = g * KH + k
                            nc.tensor.matmul(
                                ps[:, g * B:(g + 1) * B],
                                whhT[:, (kk * M + m) * 128:(kk * M + m + 1) * 128],
                                h_src[:, kk * B:(kk + 1) * B],
                                start=False, stop=(kk == KH - 1),
                            )
                    # tail for chunk k: af = sigmoid over all 4 blocks;
                    # g~ = sigmoid(2*g_pre) (host pre-scaled), so
                    # i*g = 2*(g~ - 0.5)*i and c = 2*q + f*c_prev.
                    af = af_all[:, k * 4 * B:(k + 1) * 4 * B]
                    nc.scalar.activation(af, ps[:], sig)
                    q = ap.tile([128, B], f32, tag="q")
                    nc.vector.scalar_tensor_tensor(
                        q[:], af[:, 3 * B:4 * B], 0.5, af[:, 0:B],
                        op0=mybir.AluOpType.subtract, op1=mybir.AluOpType.mult)
                    fc = ap.tile([128, B], f32, tag="fc")
                    nc.gpsimd.tensor_mul(fc[:], af[:, B:2 * B],
                                         c_prev[:, k * B:(k + 1) * B])
                    nc.vector.scalar_tensor_tensor(
                        c_t[:, k * B:(k + 1) * B], q[:], 2.0, fc[:],
                        op0=mybir.AluOpType.mult, op1=mybir.AluOpType.add)
                    if k % 2 == 1:
                        # paired tanh(c) + h = o * th over chunks k-1, k
                        th = ap.tile([128, 2 * B], f32, tag="th")
                        nc.scalar.activation(
                            th[:], c_t[:, (k - 1) * B:(k + 1) * B], tanh)
                        ho = ob.rearrange("p (s2 k2 b) -> p s2 k2 b",
                                          s2=WIN, k2=KH)
                        nc.vector.tensor_mul(
                            ho[:, s, k - 1:k + 1, :],
                            th.rearrange("p (k2 b) -> p k2 b", k2=2),
                            af4[:, k - 1:k + 1, 2, :],
                        )
                    if k == 0 and w + 2 < n_win:
                        # PE filler while the previous step's late h
                        # chunks finish: phase C for window w+2
                        phase_c_subunit(w + 2, s)
                h_src = ob[:, s * KH * B:(s + 1) * KH * B]
                c_prev = c_t
                if s == WIN - 1:
                    nc.gpsimd.dma_start(out_d[w], ob[:])

    return nc


def _get_nc(t_scan):
    key = t_scan
    if key not in _BUILT:
        _BUILT[key] = _build(t_scan)
    return _BUILT[key]


_RUNNERS = {}


def _make_runner(t_scan):
    """Compile once, return a callable in_maps -> list[dict] that only
    executes (PJRT executable cached across kernel() calls)."""
    import jax
    import jax.numpy as jnp
    import numpy as np
    from jax.sharding import Mesh, PartitionSpec
    from jax.experimental.shard_map import shard_map
    from concourse import bass2jax, mybir
    from concourse.bass2jax import _bass_exec_p, install_neuronx_cc_hook

    install_neuronx_cc_hook()
    nc = _get_nc(t_scan)
    assert nc.dbg_addr is None
    n_cores = N_CORES
    partition_name = (nc.partition_id_tensor.name
                      if nc.partition_id_tensor else None)
    in_names, out_names, out_avals, zero_shapes = [], [], [], []
    for alloc in nc.m.functions[0].allocations:
        if not isinstance(alloc, mybir.MemoryLocationSet):
            continue
        name = alloc.memorylocations[0].name
        if alloc.kind == "ExternalInput":
            if name != partition_name:
                in_names.append(name)
        elif alloc.kind == "ExternalOutput":
            shape = tuple(alloc.tensor_shape)
            npdt = mybir.dt.np(alloc.dtype)
            out_avals.append(jax.core.ShapedArray(shape, npdt))
            out_names.append(name)
            zero_shapes.append((shape, npdt))
    n_params = len(in_names)
    n_outs = len(out_names)
    all_in = in_names + out_names
    if partition_name is not None:
        all_in = all_in + [partition_name]

    def _body(*args):
        operands = list(args)
        if partition_name is not None:
            operands.append(bass2jax.partition_id_tensor())
        outs = _bass_exec_p.bind(
            *operands,
            out_avals=tuple(out_avals),
            in_names=tuple(all_in),
            out_names=tuple(out_names),
            lowering_input_output_aliases=(),
            sim_require_finite=True,
            sim_require_nnan=True,
            nc=nc,
        )
        return tuple(outs)

    devices = jax.devices()[:n_cores]
    mesh = Mesh(np.asarray(devices), ("core",))
    donate = tuple(range(n_params, n_params + n_outs))
    sharded = jax.jit(
        shard_map(_body, mesh=mesh,
                  in_specs=(PartitionSpec("core"),) * (n_params + n_outs),
                  out_specs=(PartitionSpec("core"),) * n_outs,
                  check_rep=False),
        donate_argnums=donate, keep_unused=True,
    )

    def run(in_maps):
        concat_in = [
            np.concatenate([np.asarray(m[name]) for m in in_maps], axis=0)
            for name in in_names
        ]
        concat_zeros = [
            jnp.zeros((n_cores * s[0], *s[1:]), dt) for s, dt in zero_shapes
        ]
        out_arrs = sharded(*concat_in, *concat_zeros)
        return [
            {name: np.asarray(out_arrs[i]).reshape(
                n_cores, *out_avals[i].shape)[c]
             for i, name in enumerate(out_names)}
            for c in range(n_cores)
        ]

    run.in_names = in_names
    run.out_names = out_names
    run.zero_shapes = zero_shapes
    run.sharded = sharded
    run.n_cores = n_cores
    return run


def _get_runner(t_scan):
    if t_scan not in _RUNNERS:
        _RUNNERS[t_scan] = _make_runner(t_scan)
    return _RUNNERS[t_scan]


_GATE_PERM = None


def _gate_perm():
    global _GATE_PERM
    if _GATE_PERM is None:
        # reference gate row order i,f,g,o -> kernel order i,f,o,g
        _GATE_PERM = np.concatenate([
            np.arange(0, H), np.arange(H, 2 * H),
            np.arange(3 * H, 4 * H), np.arange(2 * H, 3 * H)])
    return _GATE_PERM


def make_in_maps(x, W_ih_f, W_hh_f, b_f, W_ih_b, W_hh_b, b_b):
    """Host-side input prep: one map per core (0=forward, 1=backward)."""
    x = np.asarray(x, dtype=np.float32)
    perm = _gate_perm()
    in_maps = []
    ident = np.eye(128, dtype=np.float16)
    for d, (wih, whh, bb) in enumerate(
            [(W_ih_f, W_hh_f, b_f), (W_ih_b, W_hh_b, b_b)]):
        wihp = np.asarray(wih, np.float32)[perm].copy()
        whhp = np.asarray(whh, np.float32)[perm].copy()
        bp = np.asarray(bb, np.float32)[perm].copy()
        # pre-scale g-gate rows by 2: tanh(x) = 2*sigmoid(2x) - 1
        wihp[3 * H:] *= 2.0
        whhp[3 * H:] *= 2.0
        bp[3 * H:] *= 2.0
        xd = x if d == 0 else x[:, ::-1]
        xt = np.ascontiguousarray(
            xd.transpose(2, 1, 0).reshape(I, TB)).astype(np.float16)
        in_maps.append({
            "xT": xt,
            "wihT": np.ascontiguousarray(wihp.T).astype(np.float16),
            "whhT": np.ascontiguousarray(whhp.T).astype(np.float16),
            "bsb": np.ascontiguousarray(bp.reshape(M, 128).T),
            "ident": ident,
        })
    return in_maps


def unscramble(results, t_scan):
    """results: list of per-core out dicts -> full [32, t_scan, 1024]."""
    n_win = t_scan // WIN
    halves = []
    for d in range(2):
        raw = np.asarray(results[d]["out_raw"])  # [n_win, 128, WIN*KH*B]
        h = raw.reshape(n_win, 128, WIN, KH, B)
        h = np.ascontiguousarray(h.transpose(4, 0, 2, 3, 1))
        h = h.reshape(B, n_win * WIN, H)[:, :t_scan]
        if d == 1:
            h = h[:, ::-1]
        halves.append(h)
    return np.concatenate(halves, axis=2).astype(np.float32)


def kernel(x, W_ih_f, W_hh_f, b_f, W_ih_b, W_hh_b, b_b, _t_scan=T_SCAN):
    in_maps = make_in_maps(x, W_ih_f, W_hh_f, b_f, W_ih_b, W_hh_b, b_b)
    try:
        runner = _get_runner(_t_scan)
        results = runner(in_maps)
    except Exception:
        from concourse.bass_utils import run_bass_kernel_spmd
        res = run_bass_kernel_spmd(_get_nc(_t_scan), in_maps,
                                   list(range(N_CORES)))
        results = res.results
    return unscramble(results, _t_scan)
